# revision 1
# baseline (speedup 1.0000x reference)
"""Multi-head attention, tensor-parallel across 8 Trainium2 NeuronCores.

Sharding: core = (batch b, head-group g) with b in {0,1}, g in {0..3}.
Each core computes 4 heads (a 256-wide slice of the head dimension) for one
batch element:
  Q^T/K^T = Wq/Wk slice^T-projections of query/key (kept transposed: [dh, s])
  V       = value @ Wv slice (natural [s, dh]), with an appended ones column
  S^T     = K^T-chunk.T @ Q^T-chunk per head  -> scores transposed [j, i]
  E       = exp(S^T * scale)                  (no max subtraction; scores ~N(0,1))
  [O^T;Z] = V'.T @ E  accumulated over j      (ones column yields Z = sum_j E)
  Onorm^T = O^T * (1/Z) broadcast
  outT    = Wo-slice.T @ Onorm^T (+ bo on group-0 cores only)
Host: transposes activations into [D, S] per core, and sums the 4 group
partials per batch (the "all-reduce" of the output projection), then
transposes back.

Inputs arrive full-size; all sharding is internal.
"""

import numpy as np

# Problem shape (hardcoded per the harness contract).
B, S, D, H = 2, 2048, 1024, 16
DK = D // H              # 64 head dim
N_CORES = 8
GROUPS = N_CORES // B    # 4 head-groups
DH = D // GROUPS         # 256 head-dims per core (4 heads)
H_CORE = DH // DK        # 4 heads per core
SCALE = 1.0 / float(np.sqrt(DK))

P = 128                  # SBUF/PSUM partitions
SC = 512                 # matmul moving-dim chunk (one PSUM bank of fp32)
IB = 1024                # flash i-block (exp granule)


def build_nc(S=S, D=D, DH=DH, DK=DK, scale=SCALE, ib=IB, dtype="f32r"):
    """Build the per-core Bass module (same NEFF for all 8 cores)."""
    import concourse.bacc as bacc
    import concourse.mybir as mybir
    import concourse.tile as tile

    f32 = mybir.dt.float32
    f32r = mybir.dt.float32r
    bf16 = mybir.dt.bfloat16
    Exp = mybir.ActivationFunctionType.Exp

    KT = D // P                    # contraction tiles for projections
    NSC = S // SC                  # s chunks
    HC = DH // P                   # head-dim chunks (2)
    HPC = P // DK                  # heads per chunk (2)
    H_CORE = DH // DK
    JT = S // P                    # j tiles
    NIB = S // ib                  # i blocks
    ICB = ib // SC                 # i chunks per block
    NOUT = D // P                  # output row chunks

    cdt = {"f32r": f32r, "bf16": bf16, "f32": f32}[dtype]

    def mm(ap):
        return ap

    nc = bacc.Bacc("TRN2", target_bir_lowering=False, debug=False)

    qT = nc.dram_tensor("qT", [D, S], cdt, kind="ExternalInput")
    kTd = nc.dram_tensor("kTd", [D, S], cdt, kind="ExternalInput")
    vT = nc.dram_tensor("vT", [D, S], cdt, kind="ExternalInput")
    wq = nc.dram_tensor("wq", [D, DH], cdt, kind="ExternalInput")
    wk = nc.dram_tensor("wk", [D, DH], cdt, kind="ExternalInput")
    wv = nc.dram_tensor("wv", [D, DH], cdt, kind="ExternalInput")
    wo = nc.dram_tensor("wo", [DH, D], cdt, kind="ExternalInput")
    bq = nc.dram_tensor("bq", [P, HC], f32, kind="ExternalInput")
    bk = nc.dram_tensor("bk", [P, HC], f32, kind="ExternalInput")
    bvb = nc.dram_tensor("bvb", [P, H_CORE, DK], f32, kind="ExternalInput")
    bo = nc.dram_tensor("bo", [P, NOUT], f32, kind="ExternalInput")
    outT = nc.dram_tensor("outT", [D, S], f32, kind="ExternalOutput")

    with tile.TileContext(nc) as tc:
        with (
            tc.tile_pool(name="const", bufs=1) as cpool,
            tc.tile_pool(name="pers", bufs=1) as pers,
            tc.tile_pool(name="stream", bufs=1) as stream,
            tc.tile_pool(name="psum", bufs=1, space="PSUM") as psum,
            tc.tile_pool(name="dscratch", bufs=1, space="DRAM") as dscratch,
        ):
            # ---- constants ----
            wq_sb = cpool.tile([P, KT, DH], cdt, name="wq_sb")
            wk_sb = cpool.tile([P, KT, DH], cdt, name="wk_sb")
            wv_sb = cpool.tile([P, KT, DH], cdt, name="wv_sb")
            wo_sb = cpool.tile([P, HC, D], cdt, name="wo_sb")
            bq_sb = cpool.tile([P, HC], f32, name="bq_sb")
            bk_sb = cpool.tile([P, HC], f32, name="bk_sb")
            bvb_sb = cpool.tile([P, H_CORE, DK], f32, name="bvb_sb")
            bo_sb = cpool.tile([P, NOUT], f32, name="bo_sb")
            nc.sync.dma_start(wq_sb[:], qT_ap_rearr(wq, P))
            nc.sync.dma_start(wk_sb[:], qT_ap_rearr(wk, P))
            nc.sync.dma_start(wv_sb[:], qT_ap_rearr(wv, P))
            nc.sync.dma_start(wo_sb[:], wo[:, :].rearrange("(c p) n -> p c n", p=P))
            nc.sync.dma_start(bq_sb[:], bq[:, :])
            nc.sync.dma_start(bk_sb[:], bk[:, :])
            nc.sync.dma_start(bvb_sb[:], bvb[:, :, :])
            nc.sync.dma_start(bo_sb[:], bo[:, :])

            # ---- persistent activations ----
            # Q^T/K^T live per head on partitions 64-127 (base-64 K=64
            # matmuls sustain full rate; base-0 ones run at half rate).
            qt_h = [pers.tile([P, S], cdt, name=f"qth{h}")
                    for h in range(H_CORE)]
            kt_h = [pers.tile([P, S], cdt, name=f"kth{h}")
                    for h in range(H_CORE)]
            v_c = [pers.tile([P, JT, HPC, DK + 1], cdt, name=f"v{c}") for c in range(HC)]
            on_c = [pers.tile([P, S], cdt, name=f"on{c}") for c in range(HC)]

            for c in range(HC):
                ones_ap = v_c[c][:, :, :, DK:DK + 1]
                if dtype == "f32r":
                    ones_ap = ones_ap.bitcast(f32)
                nc.vector.memset(ones_ap, 1.0)

            # ---- projections ----
            def qk_proj(src, w_sb, b_sb, dst, chunks):
                for si in range(NSC):
                    ins = []
                    for kt in range(KT):
                        t = stream.tile([P, SC], cdt, tag="instream", bufs=12,
                                        name=f"in_{src.name}_{si}_{kt}_{chunks[0]}")
                        nc.sync.dma_start(
                            t[:], src[kt * P:(kt + 1) * P,
                                      si * SC:(si + 1) * SC])
                        ins.append(t)
                        yield
                    for c in chunks:
                        ps = psum.tile([P, SC], f32, tag="mm", bufs=4,
                                       name=f"ps_{src.name}_{si}_{c}")
                        for kt in range(KT):
                            nc.tensor.matmul(
                                ps[:],
                                lhsT=mm(w_sb[:, kt, c * P:(c + 1) * P]),
                                rhs=mm(ins[kt][:]),
                                start=(kt == 0), stop=(kt == KT - 1))
                            yield
                        stg = stream.tile([P, SC], cdt, tag="pstage", bufs=3,
                                          name=f"stg_{src.name}_{si}_{c}")
                        nc.vector.tensor_add(
                            stg[:], ps[:],
                            b_sb[:, c:c + 1].to_broadcast((P, SC)))
                        ssl = slice(si * SC, (si + 1) * SC)
                        nc.sync.dma_start(dst[c * HPC][DK:P, ssl],
                                          stg[0:DK, :])
                        nc.sync.dma_start(dst[c * HPC + 1][DK:P, ssl],
                                          stg[DK:P, :])
                        yield

            for g in (qk_proj(qT, wq_sb, bq_sb, qt_h, tuple(range(HC))),
                      qk_proj(kTd, wk_sb, bk_sb, kt_h, tuple(range(HC)))):
                for _ in g:
                    pass
            deferred = iter(())

            # V natural: psum[s, dh] = sum_k vT[k, s] * Wv[k, dh]
            for si in range(NSC):
                ins = []
                for kt in range(KT):
                    t = stream.tile([P, SC], cdt, tag="instream", bufs=12,
                                    name=f"in_v_{si}_{kt}")
                    nc.sync.dma_start(
                        t[:], vT[kt * P:(kt + 1) * P, si * SC:(si + 1) * SC])
                    ins.append(t)
                for sub in range(SC // P):
                    jt_idx = si * (SC // P) + sub
                    ps = psum.tile([P, DH], f32, tag="mm", bufs=4,
                                   name=f"ps_v_{jt_idx}")
                    for kt in range(KT):
                        nc.tensor.matmul(
                            ps[:],
                            lhsT=mm(ins[kt][:, sub * P:(sub + 1) * P]),
                            rhs=mm(wv_sb[:, kt, :]),
                            start=(kt == 0), stop=(kt == KT - 1))
                    for c in range(HC):
                        nc.vector.tensor_add(
                            v_c[c][:, jt_idx, :, 0:DK],
                            ps[:, c * P:(c + 1) * P].rearrange(
                                "p (h d) -> p h d", d=DK),
                            bvb_sb[:, c * HPC:(c + 1) * HPC, :])

            # ---- attention (flash over j, scores transposed) ----
            # Per-head blocks; sc has two buffers so scores(jt+1) overlap
            # exp(jt). AV matmuls trail one j-step so the PE program never
            # blocks the ACT engine behind unready work.
            for h in range(H_CORE):
                hc = h // HPC
                hh = h % HPC
                p0 = hh * DK
                for ibx in range(NIB):
                    i0 = ibx * ib
                    avs = [
                        psum.tile([P, SC], f32, tag="mm", bufs=4,
                                  name=f"av_{h}_{ibx}_{ic}")
                        for ic in range(ICB)
                    ]
                    e_ts = {}
                    for jt in range(JT + 1):
                        if jt < JT:
                            sc_t = psum.tile([P, ib], f32, tag="sc",
                                             bufs=2,
                                             name=f"sc_{h}_{ibx}_{jt}")
                            for ic in range(ICB):
                                nc.tensor.matmul(
                                    sc_t[:, ic * SC:(ic + 1) * SC],
                                    lhsT=mm(kt_h[h][DK:P,
                                                    jt * P:(jt + 1) * P]),
                                    rhs=mm(qt_h[h][DK:P,
                                                   i0 + ic * SC:i0 + (ic + 1) * SC]),
                                    start=True, stop=True)
                            e_t = stream.tile([P, ib], cdt, tag="e", bufs=3,
                                              name=f"e_{h}_{ibx}_{jt}")
                            nc.scalar.activation(e_t[:], sc_t[:], Exp,
                                                 bias=0.0, scale=scale)
                            e_ts[jt] = e_t
                        if jt >= 1:
                            pj = jt - 1
                            e_t = e_ts.pop(pj)
                            for ic in range(ICB):
                                nc.tensor.matmul(
                                    avs[ic][0:DK + 1, :],
                                    lhsT=mm(v_c[hc][:, pj, hh, :]),
                                    rhs=mm(e_t[:, ic * SC:(ic + 1) * SC]),
                                    start=(pj == 0), stop=(pj == JT - 1))
                    # drain AV psums to SBUF, normalize in the background
                    for ic in range(ICB):
                        av = avs[ic]
                        av_sb = stream.tile([P, SC], f32, tag="avsb", bufs=4,
                                            name=f"avsb_{h}_{ibx}_{ic}")
                        nc.vector.tensor_copy(av_sb[0:DK + 1, :],
                                              av[0:DK + 1, :])
                        rz = stream.tile([P, SC], f32, tag="rz", bufs=2,
                                         name=f"rz_{h}_{ibx}_{ic}")
                        nc.vector.reciprocal(rz[DK:DK + 1, :],
                                             av_sb[DK:DK + 1, :])
                        rz_d = dscratch.tile([1, SC], f32, tag="rzd", bufs=2,
                                             name=f"rzd_{h}_{ibx}_{ic}")
                        nc.sync.dma_start(rz_d[:], rz[DK:DK + 1, :])
                        rzb = stream.tile([P, SC], f32, tag="rzb", bufs=2,
                                          name=f"rzb_{h}_{ibx}_{ic}")
                        nc.sync.dma_start(
                            rzb[0:DK, :],
                            rz_d[:, :].to_broadcast((DK, SC)))
                        ot = stream.tile([P, SC], cdt, tag="ot", bufs=2,
                                         name=f"ot_{h}_{ibx}_{ic}")
                        nc.vector.tensor_mul(ot[0:DK, :], av_sb[0:DK, :],
                                             rzb[0:DK, :])
                        nc.sync.dma_start(
                            on_c[hc][p0:p0 + DK,
                                     i0 + ic * SC:i0 + (ic + 1) * SC],
                            ot[0:DK, :])

            # ---- output projection ----
            Ident = mybir.ActivationFunctionType.Identity
            for n in range(NOUT):
                for i in range(NSC):
                    idx = n * NSC + i
                    ps = psum.tile([P, SC], f32, tag=("sc", "mm")[idx % 2],
                                   bufs=(2, 4)[idx % 2],
                                   name=f"ps_o_{n}_{i}")
                    for c in range(HC):
                        nc.tensor.matmul(
                            ps[:],
                            lhsT=mm(wo_sb[:, c, n * P:(n + 1) * P]),
                            rhs=mm(on_c[c][:, i * SC:(i + 1) * SC]),
                            start=(c == 0), stop=(c == HC - 1))
                    o_sb = stream.tile([P, SC], f32, tag="osb", bufs=4,
                                       name=f"o_sb_{n}_{i}")
                    if idx % 2 == 0:
                        nc.scalar.activation(o_sb[:], ps[:], Ident,
                                             bias=bo_sb[:, n:n + 1],
                                             scale=1.0)
                    else:
                        nc.vector.tensor_add(
                            o_sb[:], ps[:],
                            bo_sb[:, n:n + 1].to_broadcast((P, SC)))
                    nc.sync.dma_start(
                        outT[n * P:(n + 1) * P, i * SC:(i + 1) * SC], o_sb[:])

    nc.finalize()
    return nc


def qT_ap_rearr(w_dram, p):
    """[D, N] dram weight -> [P, D//P, N] AP for SBUF load."""
    return w_dram[:, :].rearrange("(ko p) n -> p ko n", p=p)


def make_in_maps(query, key, value, Wq, bq, Wk, bk, Wv, bv, Wo, bo,
                 dtype="f32r"):
    """Shard full inputs into the 8 per-core input dicts."""
    f = lambda a: np.ascontiguousarray(np.asarray(a, dtype=np.float32))
    HC = DH // P
    NOUT = D // P
    query, key, value = f(query), f(key), f(value)
    Wq, Wk, Wv, Wo = f(Wq), f(Wk), f(Wv), f(Wo)
    bq, bk, bv, bo = f(bq), f(bk), f(bv), f(bo)
    if dtype == "bf16":
        import ml_dtypes
        cvt = lambda a: np.ascontiguousarray(a.astype(ml_dtypes.bfloat16))
    else:
        cvt = np.ascontiguousarray
    in_maps = []
    for core in range(N_CORES):
        b, g = core // GROUPS, core % GROUPS
        sl = slice(g * DH, (g + 1) * DH)
        in_maps.append({
            "qT": cvt(query[b].T),
            "kTd": cvt(key[b].T),
            "vT": cvt(value[b].T),
            "wq": cvt(Wq[:, sl]),
            "wk": cvt(Wk[:, sl]),
            "wv": cvt(Wv[:, sl]),
            "wo": cvt(Wo[sl, :]),
            "bq": np.ascontiguousarray(bq[sl].reshape(HC, P).T),
            "bk": np.ascontiguousarray(bk[sl].reshape(HC, P).T),
            "bvb": np.ascontiguousarray(
                np.broadcast_to(bv[sl].reshape(H_CORE, DK)[None], (P, H_CORE, DK))),
            "bo": (np.ascontiguousarray(bo.reshape(NOUT, P).T)
                   if g == 0 else np.zeros((P, NOUT), np.float32)),
        })
    return in_maps


# test hooks (ignored by the harness)
TRACE = False
LAST_RESULT = None
DTYPE = "bf16"
_NC_CACHE = {}


def kernel(query, key, value, Wq, bq, Wk, bk, Wv, bv, Wo, bo):
    global LAST_RESULT
    from concourse.bass_utils import run_bass_kernel_spmd

    if DTYPE not in _NC_CACHE:
        _NC_CACHE[DTYPE] = build_nc(dtype=DTYPE)
    nc = _NC_CACHE[DTYPE]

    in_maps = make_in_maps(query, key, value, Wq, bq, Wk, bk, Wv, bv, Wo, bo,
                           dtype=DTYPE)
    kwargs = {}
    if TRACE:
        kwargs = dict(trace=True, trace_cores=[0])
    res = run_bass_kernel_spmd(nc, in_maps, core_ids=list(range(N_CORES)), **kwargs)
    LAST_RESULT = res

    out = np.zeros((B, S, D), np.float32)
    for core in range(N_CORES):
        b = core // GROUPS
        out[b] += res.results[core]["outT"].T
    return out



# revision 7
# speedup vs baseline: 1.2951x; 1.2951x over previous
"""Multi-head attention, tensor-parallel across 8 Trainium2 NeuronCores.

Sharding: core = (batch b, head-group g) with b in {0,1}, g in {0..3}.
Each core computes 4 heads (a 256-wide slice of the head dimension) for one
batch element.

Schedule (single software-pipelined stream, ACT-bound steady state):
  - K projection runs kt-outer so PE tracks the kT input DMA arrival;
    8 psum chunk accumulators live in the sc/av psum slots before the
    attention rotation starts.
  - Q projection (first half of S) likewise; attention scores+exp begin
    ~23us in.  The rest of Q, all of V, and the ib0 output projection are
    interleaved into the attention stream as PE fillers.
  - Attention: per step (ibx, h, jt): 2 score matmuls (K=64, base-64) into
    a double-buffered [128,1024] psum tile, one exp on ACT, and AV matmuls
    trailing by a per-block lag (so V production can stay ahead early on).
    AV accumulates [65,1024] per block with an appended ones-column giving
    the softmax denominator Z as row 64.
  - Normalization per block: 1/Z via reciprocal_approx_fast, DRAM
    round-trip broadcast to 64 partitions, single fused psum-multiply.
  - Output projection per ibx: pairs of [128,1024] psum tiles borrowed
    from the score rotation; ib0 during ib1's attention, ib1 in the tail.

Inputs arrive full-size; all sharding is internal; host pre-arranges
weight/value layouts so every device DMA is contiguous per partition.
"""

import numpy as np

# Problem shape (hardcoded per the harness contract).
B, S, D, H = 2, 2048, 1024, 16
DK = D // H              # 64 head dim
N_CORES = 8
GROUPS = N_CORES // B    # 4 head-groups
DH = D // GROUPS         # 256 head-dims per core (4 heads)
H_CORE = DH // DK        # 4 heads per core
SCALE = 1.0 / float(np.sqrt(DK))

P = 128                  # SBUF/PSUM partitions
SC = 512                 # matmul moving-dim chunk (one PSUM bank of fp32)
IB = 1024                # i-block (exp granule, AV accumulator width)

KT = D // P              # 8 contraction tiles for projections
NSC = S // SC            # 4 s chunks
HC = DH // P             # 2 head-dim chunks per core
HPC = P // DK            # 2 heads per chunk
JT = S // P              # 16 j tiles
JP = S // (2 * P)        # 8 j pairs (V column-tile granularity)
NIB = S // IB            # 2 i blocks
NOUT = D // P            # 8 output row chunks

# per-block AV lag (steps the AV stream trails the score/exp stream);
# large early so V-projection fillers stay ahead of AV consumption.
LAGS = [17, 14, 11, 8, 6, 4, 4, 4]
E_BUFS = 20


def build_nc(dtype="bf16"):
    """Build the per-core Bass module (same NEFF for all 8 cores)."""
    import concourse.bacc as bacc
    import concourse.mybir as mybir
    import concourse.tile as tile

    f32 = mybir.dt.float32
    bf16 = mybir.dt.bfloat16
    Exp = mybir.ActivationFunctionType.Exp
    cdt = bf16

    nc = bacc.Bacc("TRN2", target_bir_lowering=False, debug=False)

    qT = nc.dram_tensor("qT", [D, S], cdt, kind="ExternalInput")
    kTd = nc.dram_tensor("kTd", [D, S], cdt, kind="ExternalInput")
    v_arr = nc.dram_tensor("v_arr", [P, JP, KT, 2 * P], cdt, kind="ExternalInput")
    wq = nc.dram_tensor("wq", [P, KT, DH], cdt, kind="ExternalInput")
    wk = nc.dram_tensor("wk", [P, KT, DH], cdt, kind="ExternalInput")
    wv = nc.dram_tensor("wv", [P, KT, DH], cdt, kind="ExternalInput")
    wo = nc.dram_tensor("wo", [P, HC, D], cdt, kind="ExternalInput")
    bq = nc.dram_tensor("bq", [P, HC], f32, kind="ExternalInput")
    bk = nc.dram_tensor("bk", [P, HC], f32, kind="ExternalInput")
    bvb = nc.dram_tensor("bvb", [P, H_CORE, DK], f32, kind="ExternalInput")
    bo = nc.dram_tensor("bo", [P, NOUT], f32, kind="ExternalInput")
    outT = nc.dram_tensor("outT", [D, S], cdt, kind="ExternalOutput")

    with tile.TileContext(nc) as tc:
        with (
            tc.tile_pool(name="const", bufs=1) as cpool,
            tc.tile_pool(name="pers", bufs=1) as pers,
            tc.tile_pool(name="stream", bufs=1) as stream,
            tc.tile_pool(name="psum", bufs=1, space="PSUM") as psum,
            tc.tile_pool(name="dscratch", bufs=1, space="DRAM") as dscratch,
        ):
            # ---- constants (host pre-arranged; contiguous per partition) ----
            wk_sb = cpool.tile([P, KT, DH], cdt, name="wk_sb")
            wq_sb = cpool.tile([P, KT, DH], cdt, name="wq_sb")
            wv_sb = cpool.tile([P, KT, DH], cdt, name="wv_sb")
            wo_sb = cpool.tile([P, HC, D], cdt, name="wo_sb")
            bq_sb = cpool.tile([P, HC], f32, name="bq_sb")
            bk_sb = cpool.tile([P, HC], f32, name="bk_sb")
            bvb_sb = cpool.tile([P, H_CORE, DK], f32, name="bvb_sb")
            bo_sb = cpool.tile([P, NOUT], f32, name="bo_sb")

            # ---- persistent activations ----
            # Q^T/K^T per head on partitions 64-127 (base-64 K=64 matmuls
            # sustain full rate).
            qt_h = [pers.tile([P, S], cdt, name=f"qth{h}") for h in range(H_CORE)]
            kt_h = [pers.tile([P, S], cdt, name=f"kth{h}") for h in range(H_CORE)]
            v_c = [pers.tile([P, JT, HPC, DK + 1], cdt, name=f"v{c}")
                   for c in range(HC)]
            on_c = [pers.tile([P, S], cdt, name=f"on{c}") for c in range(HC)]

            def sc_tile(name):
                return psum.tile([P, IB], f32, tag="sc", bufs=2, name=name)

            def av_tile(name):
                return psum.tile([P, IB], f32, tag="av", bufs=2, name=name)

            # ---- DMA issue: weights + inputs, in consumption order ----
            nc.sync.dma_start(wk_sb[:], wk[:, :, :])
            nc.sync.dma_start(bk_sb[:], bk[:, :])
            kin = []
            for kt in range(KT):
                t = stream.tile([P, 2 * S // 2], cdt, tag="kin", bufs=KT,
                                name=f"kin{kt}")
                nc.sync.dma_start(t[:], kTd[kt * P:(kt + 1) * P, :])
                kin.append(t)
            nc.sync.dma_start(wq_sb[:], wq[:, :, :])
            nc.sync.dma_start(bq_sb[:], bq[:, :])
            qin = []
            for kt in range(KT):
                t = stream.tile([P, S // 2], cdt, tag="qin", bufs=KT,
                                name=f"qin01_{kt}")
                nc.sync.dma_start(t[:], qT[kt * P:(kt + 1) * P, 0:S // 2])
                qin.append(t)
            nc.sync.dma_start(wv_sb[:], wv[:, :, :])
            nc.sync.dma_start(bvb_sb[:], bvb[:, :, :])
            vin = []
            for jp in range(JP):
                t = stream.tile([P, KT, 2 * P], cdt, tag="kin", bufs=KT,
                                name=f"vin{jp}")
                nc.sync.dma_start(t[:], v_arr[:, jp, :, :])
                vin.append(t)
            nc.sync.dma_start(wo_sb[:], wo[:, :, :])
            nc.sync.dma_start(bo_sb[:], bo[:, :])

            for c in range(HC):
                nc.gpsimd.memset(v_c[c][:, :, :, DK:DK + 1], 1.0)

            # ---- K projection: kt-outer, 8 chunk accumulators ----
            # chunk (c, si) -> tile (c*2 + si//2), half si%2
            kboot = [sc_tile("kb0"), sc_tile("kb1"), av_tile("kb2"),
                     av_tile("kb3")]
            for kt in range(KT):
                for c in range(HC):
                    for si in range(NSC):
                        dst = kboot[c * 2 + si // 2]
                        half = si % 2
                        nc.tensor.matmul(
                            dst[:, half * SC:(half + 1) * SC],
                            lhsT=wk_sb[:, kt, c * P:(c + 1) * P],
                            rhs=kin[kt][:, si * SC:(si + 1) * SC],
                            start=(kt == 0), stop=(kt == KT - 1))
            for c in range(HC):
                stg = stream.tile([P, S], cdt, tag="stg", bufs=2,
                                  name=f"stgk{c}")
                for half in range(2):
                    nc.vector.tensor_scalar_add(
                        stg[:, half * IB:(half + 1) * IB],
                        kboot[c * 2 + half][:, :], bk_sb[:, c:c + 1])
                nc.sync.dma_start(kt_h[c * HPC][DK:P, :], stg[0:DK, :])
                nc.sync.dma_start(kt_h[c * HPC + 1][DK:P, :], stg[DK:P, :])

            # ---- Q projection, si 0-1 (i cols 0:1024) ----
            qboot = [sc_tile("qb0"), av_tile("qb1")]  # tile c, halves si
            for kt in range(KT):
                for c in range(HC):
                    for si in range(2):
                        nc.tensor.matmul(
                            qboot[c][:, si * SC:(si + 1) * SC],
                            lhsT=wq_sb[:, kt, c * P:(c + 1) * P],
                            rhs=qin[kt][:, si * SC:(si + 1) * SC],
                            start=(kt == 0), stop=(kt == KT - 1))
            for c in range(HC):
                stg = stream.tile([P, S], cdt, tag="stg", bufs=2,
                                  name=f"stgq01_{c}")
                nc.vector.tensor_scalar_add(stg[:, 0:IB], qboot[c][:, :],
                                            bq_sb[:, c:c + 1])
                nc.sync.dma_start(qt_h[c * HPC][DK:P, 0:IB], stg[0:DK, 0:IB])
                nc.sync.dma_start(qt_h[c * HPC + 1][DK:P, 0:IB],
                                  stg[DK:P, 0:IB])

            # second-half Q input loads (reuse qin slots; WAR on q01 chains)
            qin23 = []
            for kt in range(KT):
                t = stream.tile([P, S // 2], cdt, tag="qin", bufs=KT,
                                name=f"qin23_{kt}")
                nc.sync.dma_start(t[:], qT[kt * P:(kt + 1) * P, S // 2:S])
                qin23.append(t)

            # ---- filler emitters (run between attention steps) ----
            def v_chain(jt):
                """One V j-tile: 8 matmuls accumulating [P,256] in its own
                psum bank (a start-flag matmul zeroes the whole 2KB bank, so
                each chain gets a full [P,SC] half), plus bias-add drains."""
                q = jt % 2
                if q == 0:
                    v_chain.cur = av_tile(f"vps{jt // 2}")
                ps = v_chain.cur[:, q * SC:q * SC + 2 * P]
                for kt in range(KT):
                    nc.tensor.matmul(
                        ps,
                        lhsT=vin[jt // 2][:, kt, (jt % 2) * P:(jt % 2 + 1) * P],
                        rhs=wv_sb[:, kt, :],
                        start=(kt == 0), stop=(kt == KT - 1))
                for c in range(HC):
                    nc.vector.tensor_add(
                        v_c[c][:, jt, :, 0:DK],
                        ps.rearrange("p (h d) -> p h d", d=DK)[
                            :, c * HPC:(c + 1) * HPC, :],
                        bvb_sb[:, c * HPC:(c + 1) * HPC, :])

            def q23_item(c):
                """Q projection chunk (c, si 2-3) in one av-slot tile."""
                ps = av_tile(f"q23_{c}")
                for kt in range(KT):
                    for si in range(2):
                        nc.tensor.matmul(
                            ps[:, si * SC:(si + 1) * SC],
                            lhsT=wq_sb[:, kt, c * P:(c + 1) * P],
                            rhs=qin23[kt][:, si * SC:(si + 1) * SC],
                            start=(kt == 0), stop=(kt == KT - 1))
                stg = stream.tile([P, S], cdt, tag="stg", bufs=2,
                                  name=f"stgq23_{c}")
                nc.vector.tensor_scalar_add(stg[:, 0:IB], ps[:, :],
                                            bq_sb[:, c:c + 1])
                nc.sync.dma_start(qt_h[c * HPC][DK:P, IB:S], stg[0:DK, 0:IB])
                nc.sync.dma_start(qt_h[c * HPC + 1][DK:P, IB:S],
                                  stg[DK:P, 0:IB])

            def out_pair(ibx, pair):
                """Output projection for rows [256*pair, 256*(pair+1)) of
                outT, i-cols of ibx: two sc-slot tiles (paired to keep the
                score rotation parity)."""
                for sub in range(2):
                    n = pair * 2 + sub
                    ps = sc_tile(f"ops{ibx}_{n}")
                    for ic in range(2):
                        for c in range(HC):
                            nc.tensor.matmul(
                                ps[:, ic * SC:(ic + 1) * SC],
                                lhsT=wo_sb[:, c, n * P:(n + 1) * P],
                                rhs=on_c[c][:, ibx * IB + ic * SC:
                                            ibx * IB + (ic + 1) * SC],
                                start=(c == 0), stop=(c == HC - 1))
                    o_sb = stream.tile([P, IB], cdt, tag="osb", bufs=3,
                                       name=f"osb{ibx}_{n}")
                    nc.vector.tensor_scalar_add(o_sb[:, :], ps[:, :],
                                                bo_sb[:, n:n + 1])
                    nc.sync.dma_start(
                        outT[n * P:(n + 1) * P, ibx * IB:(ibx + 1) * IB],
                        o_sb[:, :])

            fillers = []
            for jt in range(JT):
                fillers.append((lambda jt=jt: v_chain(jt), 0))
            for c in range(HC):
                # both q23 tiles must enter the av rotation before blk0's
                # long-lived tile so the 2-slot parity keeps every blkN+1
                # waiting on a short-lived user, not on blkN
                fillers.append((lambda c=c: q23_item(c), LAGS[0] - 1))
            for pair in range(NOUT // 2):
                fillers.append((lambda p=pair: out_pair(0, p), 80 + 7 * pair))

            # ---- attention stream ----
            steps = [(ibx, h, jt)
                     for ibx in range(NIB)
                     for h in range(H_CORE)
                     for jt in range(JT)]
            n_steps = len(steps)
            e_tiles = [None] * n_steps
            blk_av = {}

            def emit_av(s):
                ibx, h, jt = steps[s]
                blk = s // JT
                hc, hh = h // HPC, h % HPC
                if jt == 0:
                    blk_av[blk] = av_tile(f"avb{blk}")
                av = blk_av[blk]
                e_t = e_tiles[s]
                for ic in range(2):
                    nc.tensor.matmul(
                        av[0:DK + 1, ic * SC:(ic + 1) * SC],
                        lhsT=v_c[hc][:, jt, hh, :],
                        rhs=e_t[:, ic * SC:(ic + 1) * SC],
                        start=(jt == 0), stop=(jt == JT - 1))
                e_tiles[s] = None
                if jt == JT - 1:
                    emit_norm(blk, av)

            def emit_norm(blk, av):
                ibx = blk // H_CORE
                h = blk % H_CORE
                hc, hh = h // HPC, h % HPC
                rz = stream.tile([P, IB], f32, tag="rz", bufs=2,
                                 name=f"rz{blk}")
                nc.vector.reciprocal(rz[DK:DK + 1, :], av[DK:DK + 1, :])
                rz_d = dscratch.tile([1, IB], f32, tag="rzd", bufs=2,
                                     name=f"rzd{blk}")
                nc.sync.dma_start(rz_d[:], rz[DK:DK + 1, :])
                rzb = stream.tile([P, IB], f32, tag="rzb", bufs=2,
                                  name=f"rzb{blk}")
                nc.sync.dma_start(rzb[0:DK, :],
                                  rz_d[:, :].to_broadcast((DK, IB)))
                ot = stream.tile([P, IB], cdt, tag="ot", bufs=2,
                                 name=f"ot{blk}")
                nc.vector.tensor_mul(ot[0:DK, :], av[0:DK, :], rzb[0:DK, :])
                nc.sync.dma_start(
                    on_c[hc][hh * DK:(hh + 1) * DK,
                             ibx * IB:(ibx + 1) * IB],
                    ot[0:DK, :])

            av_ptr = 0
            fill_ptr = 0
            for s in range(n_steps):
                ibx, h, jt = steps[s]
                blk = s // JT
                sc = sc_tile(f"sc{s}")
                for ic in range(2):
                    nc.tensor.matmul(
                        sc[:, ic * SC:(ic + 1) * SC],
                        lhsT=kt_h[h][DK:P, jt * P:(jt + 1) * P],
                        rhs=qt_h[h][DK:P, ibx * IB + ic * SC:
                                    ibx * IB + (ic + 1) * SC],
                        start=True, stop=True)
                e_t = stream.tile([P, IB], cdt, tag="e", bufs=E_BUFS,
                                  name=f"e{s}")
                nc.scalar.activation(e_t[:], sc[:], Exp, bias=0.0,
                                     scale=SCALE)
                e_tiles[s] = e_t
                # fillers first (one per step when eligible): av-rotation
                # parity requires filler tiles to precede a block's av tile
                # allocated in the same step
                if fill_ptr < len(fillers) and fillers[fill_ptr][1] <= s:
                    fillers[fill_ptr][0]()
                    fill_ptr += 1
                # trailing AV stream
                while av_ptr < n_steps and av_ptr <= s - LAGS[av_ptr // JT]:
                    emit_av(av_ptr)
                    av_ptr += 1

            # flush
            while fill_ptr < len(fillers):
                fillers[fill_ptr][0]()
                fill_ptr += 1
            while av_ptr < n_steps:
                emit_av(av_ptr)
                av_ptr += 1
            for pair in range(NOUT // 2):
                out_pair(1, pair)

    nc.finalize()
    return nc


def make_in_maps(query, key, value, Wq, bq, Wk, bk, Wv, bv, Wo, bo,
                 dtype="bf16"):
    """Shard full inputs into the 8 per-core input dicts."""
    import ml_dtypes
    f = lambda a: np.asarray(a, dtype=np.float32)
    query, key, value = f(query), f(key), f(value)
    Wq, Wk, Wv, Wo = f(Wq), f(Wk), f(Wv), f(Wo)
    bq, bk, bv, bo = f(bq), f(bk), f(bv), f(bo)
    cvt = lambda a: np.ascontiguousarray(a.astype(ml_dtypes.bfloat16))

    def warr(W, sl):
        # [D, DH] slice -> [P, KT, DH] with w[p, kt, n] = W[kt*P+p, sl][n]
        return cvt(W[:, sl].reshape(KT, P, DH).transpose(1, 0, 2))

    in_maps = []
    for core in range(N_CORES):
        b, g = core // GROUPS, core % GROUPS
        sl = slice(g * DH, (g + 1) * DH)
        vT = value[b].T  # [D, S]
        v_arr = vT[:, :].reshape(KT, P, JP, 2 * P).transpose(1, 2, 0, 3)
        in_maps.append({
            "qT": cvt(query[b].T),
            "kTd": cvt(key[b].T),
            "v_arr": cvt(np.ascontiguousarray(v_arr)),
            "wq": warr(Wq, sl),
            "wk": warr(Wk, sl),
            "wv": warr(Wv, sl),
            "wo": cvt(Wo[sl, :].reshape(HC, P, D).transpose(1, 0, 2)),
            "bq": np.ascontiguousarray(bq[sl].reshape(HC, P).T),
            "bk": np.ascontiguousarray(bk[sl].reshape(HC, P).T),
            "bvb": np.ascontiguousarray(
                np.broadcast_to(bv[sl].reshape(H_CORE, DK)[None],
                                (P, H_CORE, DK))),
            "bo": (np.ascontiguousarray(bo.reshape(NOUT, P).T)
                   if g == 0 else np.zeros((P, NOUT), np.float32)),
        })
    return in_maps


# test hooks (ignored by the harness)
TRACE = False
LAST_RESULT = None
DTYPE = "bf16"
_NC_CACHE = {}


def kernel(query, key, value, Wq, bq, Wk, bk, Wv, bv, Wo, bo):
    global LAST_RESULT
    from concourse.bass_utils import run_bass_kernel_spmd

    if DTYPE not in _NC_CACHE:
        _NC_CACHE[DTYPE] = build_nc(dtype=DTYPE)
    nc = _NC_CACHE[DTYPE]

    in_maps = make_in_maps(query, key, value, Wq, bq, Wk, bk, Wv, bv, Wo, bo,
                           dtype=DTYPE)
    kwargs = {}
    if TRACE:
        kwargs = dict(trace=True, trace_cores=[0])
    res = run_bass_kernel_spmd(nc, in_maps, core_ids=list(range(N_CORES)),
                               **kwargs)
    LAST_RESULT = res

    out = np.zeros((B, S, D), np.float32)
    for core in range(N_CORES):
        b = core // GROUPS
        out[b] += res.results[core]["outT"].astype(np.float32).T
    return out


# revision 10
# speedup vs baseline: 1.3998x; 1.0808x over previous
"""Multi-head attention, tensor-parallel across 8 Trainium2 NeuronCores.

Sharding: core = (batch b, head-group g) with b in {0,1}, g in {0..3}.
Each core computes 4 heads (a 256-wide slice of the head dimension) for one
batch element.

Schedule (single software-pipelined stream, ACT-bound steady state):
  - K projection runs kt-outer so PE tracks the kT input DMA arrival;
    8 psum chunk accumulators live in the sc/av psum slots before the
    attention rotation starts.
  - Q projection (first half of S) likewise; attention scores+exp begin
    ~23us in.  The rest of Q, all of V, and the ib0 output projection are
    interleaved into the attention stream as PE fillers.
  - Attention: per step (ibx, h, jt): 2 score matmuls (K=64, base-64) into
    a double-buffered [128,1024] psum tile, one exp on ACT, and AV matmuls
    trailing by a per-block lag (so V production can stay ahead early on).
    AV accumulates [65,1024] per block with an appended ones-column giving
    the softmax denominator Z as row 64.
  - Normalization per block: 1/Z via reciprocal_approx_fast, DRAM
    round-trip broadcast to 64 partitions, single fused psum-multiply.
  - Output projection per ibx: pairs of [128,1024] psum tiles borrowed
    from the score rotation; ib0 during ib1's attention, ib1 in the tail.

Inputs arrive full-size; all sharding is internal; host pre-arranges
weight/value layouts so every device DMA is contiguous per partition.
"""

import numpy as np

# Problem shape (hardcoded per the harness contract).
B, S, D, H = 2, 2048, 1024, 16
DK = D // H              # 64 head dim
N_CORES = 8
GROUPS = N_CORES // B    # 4 head-groups
DH = D // GROUPS         # 256 head-dims per core (4 heads)
H_CORE = DH // DK        # 4 heads per core
SCALE = 1.0 / float(np.sqrt(DK))

P = 128                  # SBUF/PSUM partitions
SC = 512                 # matmul moving-dim chunk (one PSUM bank of fp32)
IB = 1024                # i-block (exp granule, AV accumulator width)

KT = D // P              # 8 contraction tiles for projections
NSC = S // SC            # 4 s chunks
HC = DH // P             # 2 head-dim chunks per core
HPC = P // DK            # 2 heads per chunk
JT = S // P              # 16 j tiles
JP = S // (2 * P)        # 8 j pairs (V column-tile granularity)
NIB = S // IB            # 2 i blocks
NOUT = D // P            # 8 output row chunks

# per-block AV lag (steps the AV stream trails the score/exp stream);
# large early so V-projection fillers stay ahead of AV consumption.
LAGS = [17, 14, 11, 8, 6, 4, 4, 4]
E_BUFS = 20


def build_nc(dtype="bf16"):
    """Build the per-core Bass module (same NEFF for all 8 cores)."""
    import concourse.bacc as bacc
    import concourse.mybir as mybir
    import concourse.tile as tile

    f32 = mybir.dt.float32
    bf16 = mybir.dt.bfloat16
    Exp = mybir.ActivationFunctionType.Exp
    cdt = bf16

    nc = bacc.Bacc("TRN2", target_bir_lowering=False, debug=False)

    qT = nc.dram_tensor("qT", [D, S], cdt, kind="ExternalInput")
    kTd = nc.dram_tensor("kTd", [D, S], cdt, kind="ExternalInput")
    v_arr = nc.dram_tensor("v_arr", [P, JP, KT, 2 * P], cdt, kind="ExternalInput")
    wq = nc.dram_tensor("wq", [P, KT, DH], cdt, kind="ExternalInput")
    wk = nc.dram_tensor("wk", [P, KT, DH], cdt, kind="ExternalInput")
    wv = nc.dram_tensor("wv", [P, KT, DH], cdt, kind="ExternalInput")
    wo = nc.dram_tensor("wo", [P, HC, D], cdt, kind="ExternalInput")
    bq = nc.dram_tensor("bq", [P, HC], f32, kind="ExternalInput")
    bk = nc.dram_tensor("bk", [P, HC], f32, kind="ExternalInput")
    bvb = nc.dram_tensor("bvb", [P, H_CORE, DK], f32, kind="ExternalInput")
    bo = nc.dram_tensor("bo", [P, NOUT], f32, kind="ExternalInput")
    outT = nc.dram_tensor("outT", [D, S], cdt, kind="ExternalOutput")

    with tile.TileContext(nc) as tc:
        with (
            tc.tile_pool(name="const", bufs=1) as cpool,
            tc.tile_pool(name="pers", bufs=1) as pers,
            tc.tile_pool(name="stream", bufs=1) as stream,
            tc.tile_pool(name="psum", bufs=1, space="PSUM") as psum,
            tc.tile_pool(name="dscratch", bufs=1, space="DRAM") as dscratch,
        ):
            # ---- constants (host pre-arranged; contiguous per partition) ----
            wk_sb = cpool.tile([P, KT, DH], cdt, name="wk_sb")
            wq_sb = cpool.tile([P, KT, DH], cdt, name="wq_sb")
            wv_sb = cpool.tile([P, KT, DH], cdt, name="wv_sb")
            wo_sb = cpool.tile([P, HC, D], cdt, name="wo_sb")
            bq_sb = cpool.tile([P, HC], f32, name="bq_sb")
            bk_sb = cpool.tile([P, HC], f32, name="bk_sb")
            bvb_sb = cpool.tile([P, H_CORE, DK], f32, name="bvb_sb")
            bo_sb = cpool.tile([P, NOUT], f32, name="bo_sb")

            # ---- persistent activations ----
            # Q^T/K^T per head on partitions 64-127 (base-64 K=64 matmuls
            # sustain full rate).
            qt_h = [pers.tile([P, S], cdt, name=f"qth{h}") for h in range(H_CORE)]
            kt_h = [pers.tile([P, S], cdt, name=f"kth{h}") for h in range(H_CORE)]
            v_c = [pers.tile([P, JT, HPC, DK + 1], cdt, name=f"v{c}")
                   for c in range(HC)]
            on_c = [pers.tile([P, S], cdt, name=f"on{c}") for c in range(HC)]

            def sc_tile(name):
                return psum.tile([P, IB], f32, tag="sc", bufs=2, name=name)

            def av_tile(name):
                return psum.tile([P, IB], f32, tag="av", bufs=2, name=name)

            # ---- DMA issue: weights + inputs, in consumption order ----
            # kT + qT01 stream on SP at full bandwidth; vin reuses the qin
            # slots and qin23 the kin slots, so their transfers are held (WAR)
            # until the k-/q01-projection matmuls consume the first users --
            # a free throttle that keeps the early loads uncontended.  vin /
            # qin23 issue from gpsimd so their slot-waits never block SP.
            for c in range(HC):
                nc.gpsimd.memset(v_c[c][:, :, :, DK:DK + 1], 1.0)
            nc.sync.dma_start(wk_sb[:], wk[:, :, :])
            nc.sync.dma_start(bk_sb[:], bk[:, :])
            kin = []
            for kt in range(KT):
                t = stream.tile([P, S], cdt, tag="kin", bufs=KT,
                                name=f"kin{kt}")
                nc.sync.dma_start(t[:], kTd[kt * P:(kt + 1) * P, :])
                kin.append(t)
            nc.sync.dma_start(wq_sb[:], wq[:, :, :])
            nc.sync.dma_start(bq_sb[:], bq[:, :])
            qin = []
            for kt in range(KT):
                t = stream.tile([P, S // 2], cdt, tag="qin", bufs=KT,
                                name=f"qin01_{kt}")
                nc.sync.dma_start(t[:], qT[kt * P:(kt + 1) * P, 0:S // 2])
                qin.append(t)
            nc.sync.dma_start(wv_sb[:], wv[:, :, :])
            nc.sync.dma_start(bvb_sb[:], bvb[:, :, :])
            nc.sync.dma_start(wo_sb[:], wo[:, :, :])
            nc.sync.dma_start(bo_sb[:], bo[:, :])
            vin = []
            for jp in range(JP):
                t = stream.tile([P, KT, 2 * P], cdt, tag="qin", bufs=KT,
                                name=f"vin{jp}")
                nc.gpsimd.dma_start(t[:], v_arr[:, jp, :, :])
                vin.append(t)
            qin23 = []
            for kt in range(KT):
                t = stream.tile([P, S // 2], cdt, tag="kin", bufs=KT,
                                name=f"qin23_{kt}")
                nc.gpsimd.dma_start(t[:], qT[kt * P:(kt + 1) * P, S // 2:S])
                qin23.append(t)

            # ---- K projection: kt-outer, 8 chunk accumulators ----
            # chunk (c, si) -> tile (c*2 + si//2), half si%2
            kboot = [sc_tile("kb0"), sc_tile("kb1"), av_tile("kb2"),
                     av_tile("kb3")]
            for kt in range(KT):
                for c in range(HC):
                    for si in range(NSC):
                        dst = kboot[c * 2 + si // 2]
                        half = si % 2
                        nc.tensor.matmul(
                            dst[:, half * SC:(half + 1) * SC],
                            lhsT=wk_sb[:, kt, c * P:(c + 1) * P],
                            rhs=kin[kt][:, si * SC:(si + 1) * SC],
                            start=(kt == 0), stop=(kt == KT - 1))
            for c in range(HC):
                stg = stream.tile([P, S], cdt, tag="stg", bufs=2,
                                  name=f"stgk{c}")
                for half in range(2):
                    nc.vector.tensor_scalar_add(
                        stg[:, half * IB:(half + 1) * IB],
                        kboot[c * 2 + half][:, :], bk_sb[:, c:c + 1])
                nc.sync.dma_start(kt_h[c * HPC][DK:P, :], stg[0:DK, :])
                nc.sync.dma_start(kt_h[c * HPC + 1][DK:P, :], stg[DK:P, :])

            # ---- Q projection, si 0-1 (i cols 0:1024) ----
            qboot = [sc_tile("qb0"), av_tile("qb1")]  # tile c, halves si
            for kt in range(KT):
                for c in range(HC):
                    for si in range(2):
                        nc.tensor.matmul(
                            qboot[c][:, si * SC:(si + 1) * SC],
                            lhsT=wq_sb[:, kt, c * P:(c + 1) * P],
                            rhs=qin[kt][:, si * SC:(si + 1) * SC],
                            start=(kt == 0), stop=(kt == KT - 1))
            for c in range(HC):
                stg = stream.tile([P, S], cdt, tag="stg", bufs=2,
                                  name=f"stgq01_{c}")
                nc.vector.tensor_scalar_add(stg[:, 0:IB], qboot[c][:, :],
                                            bq_sb[:, c:c + 1])
                nc.sync.dma_start(qt_h[c * HPC][DK:P, 0:IB], stg[0:DK, 0:IB])
                nc.sync.dma_start(qt_h[c * HPC + 1][DK:P, 0:IB],
                                  stg[DK:P, 0:IB])

            # ---- filler emitters (run between attention steps) ----
            def v_chain(jt):
                """One V j-tile: 8 matmuls accumulating [P,256] in its own
                psum bank (a start-flag matmul zeroes the whole 2KB bank, so
                each chain gets a full [P,SC] half), plus bias-add drains."""
                q = jt % 2
                if q == 0:
                    v_chain.cur = av_tile(f"vps{jt // 2}")
                ps = v_chain.cur[:, q * SC:q * SC + 2 * P]
                for kt in range(KT):
                    nc.tensor.matmul(
                        ps,
                        lhsT=vin[jt // 2][:, kt, (jt % 2) * P:(jt % 2 + 1) * P],
                        rhs=wv_sb[:, kt, :],
                        start=(kt == 0), stop=(kt == KT - 1))
                for c in range(HC):
                    nc.vector.tensor_add(
                        v_c[c][:, jt, :, 0:DK],
                        ps.rearrange("p (h d) -> p h d", d=DK)[
                            :, c * HPC:(c + 1) * HPC, :],
                        bvb_sb[:, c * HPC:(c + 1) * HPC, :])

            def q23_item(c):
                """Q projection chunk (c, si 2-3) in one av-slot tile."""
                ps = av_tile(f"q23_{c}")
                for kt in range(KT):
                    for si in range(2):
                        nc.tensor.matmul(
                            ps[:, si * SC:(si + 1) * SC],
                            lhsT=wq_sb[:, kt, c * P:(c + 1) * P],
                            rhs=qin23[kt][:, si * SC:(si + 1) * SC],
                            start=(kt == 0), stop=(kt == KT - 1))
                stg = stream.tile([P, S], cdt, tag="stg", bufs=2,
                                  name=f"stgq23_{c}")
                nc.vector.tensor_scalar_add(stg[:, 0:IB], ps[:, :],
                                            bq_sb[:, c:c + 1])
                nc.sync.dma_start(qt_h[c * HPC][DK:P, IB:S], stg[0:DK, 0:IB])
                nc.sync.dma_start(qt_h[c * HPC + 1][DK:P, IB:S],
                                  stg[DK:P, 0:IB])

            def out_pair(ibx, pair):
                """Output projection for rows [256*pair, 256*(pair+1)) of
                outT, i-cols of ibx: two sc-slot tiles (paired to keep the
                score rotation parity)."""
                for sub in range(2):
                    n = pair * 2 + sub
                    ps = sc_tile(f"ops{ibx}_{n}")
                    for ic in range(2):
                        for c in range(HC):
                            nc.tensor.matmul(
                                ps[:, ic * SC:(ic + 1) * SC],
                                lhsT=wo_sb[:, c, n * P:(n + 1) * P],
                                rhs=on_c[c][:, ibx * IB + ic * SC:
                                            ibx * IB + (ic + 1) * SC],
                                start=(c == 0), stop=(c == HC - 1))
                    o_sb = stream.tile([P, IB], cdt, tag="osb", bufs=3,
                                       name=f"osb{ibx}_{n}")
                    nc.vector.tensor_scalar_add(o_sb[:, :], ps[:, :],
                                                bo_sb[:, n:n + 1])
                    nc.sync.dma_start(
                        outT[n * P:(n + 1) * P, ibx * IB:(ibx + 1) * IB],
                        o_sb[:, :])

            fillers = []
            for jt in range(JT):
                fillers.append((lambda jt=jt: v_chain(jt), 0))
            for c in range(HC):
                # both q23 tiles must enter the av rotation before blk0's
                # long-lived tile so the 2-slot parity keeps every blkN+1
                # waiting on a short-lived user, not on blkN
                fillers.append((lambda c=c: q23_item(c), LAGS[0] - 1))
            for pair in range(NOUT // 2):
                fillers.append((lambda p=pair: out_pair(0, p), 80 + 7 * pair))

            # ---- attention stream ----
            steps = [(ibx, h, jt)
                     for ibx in range(NIB)
                     for h in range(H_CORE)
                     for jt in range(JT)]
            n_steps = len(steps)
            e_tiles = [None] * n_steps
            blk_av = {}

            def emit_av(s):
                ibx, h, jt = steps[s]
                blk = s // JT
                hc, hh = h // HPC, h % HPC
                if jt == 0:
                    blk_av[blk] = av_tile(f"avb{blk}")
                av = blk_av[blk]
                e_t = e_tiles[s]
                for ic in range(2):
                    nc.tensor.matmul(
                        av[0:DK + 1, ic * SC:(ic + 1) * SC],
                        lhsT=v_c[hc][:, jt, hh, :],
                        rhs=e_t[:, ic * SC:(ic + 1) * SC],
                        start=(jt == 0), stop=(jt == JT - 1))
                e_tiles[s] = None
                if jt == JT - 1:
                    emit_norm(blk, av)

            def emit_norm(blk, av):
                """Copy [O;Z] off psum (frees the av slot fast), spread the
                Z row over 64 partitions via a DRAM round trip so the
                microcoded DVE reciprocal runs on free-size 16 instead of
                1024, broadcast 1/Z back, and apply in one fused multiply."""
                ibx = blk // H_CORE
                h = blk % H_CORE
                hc, hh = h // HPC, h % HPC
                avs = stream.tile([P, IB], f32, tag="avsb", bufs=2,
                                  name=f"avsb{blk}")
                nc.vector.tensor_copy(avs[0:DK + 1, :], av[0:DK + 1, :])
                z_d = dscratch.tile([1, IB], f32, tag="zd", bufs=2,
                                    name=f"zd{blk}")
                nc.sync.dma_start(z_d[:], avs[DK:DK + 1, :])
                zq = stream.tile([P, IB // DK], f32, tag="zq", bufs=2,
                                 name=f"zq{blk}")
                nc.sync.dma_start(
                    zq[0:DK, :],
                    z_d[:, :].rearrange("a (p f) -> (a p) f", p=DK))
                zqr = stream.tile([P, IB // DK], f32, tag="zqr", bufs=2,
                                  name=f"zqr{blk}")
                nc.vector.reciprocal(zqr[0:DK, :], zq[0:DK, :])
                rz_d = dscratch.tile([1, IB], f32, tag="rzd", bufs=2,
                                     name=f"rzd{blk}")
                nc.sync.dma_start(
                    rz_d[:, :].rearrange("a (p f) -> (a p) f", p=DK),
                    zqr[0:DK, :])
                rzb = stream.tile([P, IB], f32, tag="rzb", bufs=2,
                                  name=f"rzb{blk}")
                nc.sync.dma_start(rzb[0:DK, :],
                                  rz_d[:, :].to_broadcast((DK, IB)))
                ot = stream.tile([P, IB], cdt, tag="ot", bufs=2,
                                 name=f"ot{blk}")
                nc.vector.tensor_mul(ot[0:DK, :], avs[0:DK, :], rzb[0:DK, :])
                nc.sync.dma_start(
                    on_c[hc][hh * DK:(hh + 1) * DK,
                             ibx * IB:(ibx + 1) * IB],
                    ot[0:DK, :])

            av_ptr = 0
            fill_ptr = 0
            for s in range(n_steps):
                ibx, h, jt = steps[s]
                blk = s // JT
                sc = sc_tile(f"sc{s}")
                for ic in range(2):
                    nc.tensor.matmul(
                        sc[:, ic * SC:(ic + 1) * SC],
                        lhsT=kt_h[h][DK:P, jt * P:(jt + 1) * P],
                        rhs=qt_h[h][DK:P, ibx * IB + ic * SC:
                                    ibx * IB + (ic + 1) * SC],
                        start=True, stop=True)
                e_t = stream.tile([P, IB], cdt, tag="e", bufs=E_BUFS,
                                  name=f"e{s}")
                nc.scalar.activation(e_t[:], sc[:], Exp, bias=0.0,
                                     scale=SCALE)
                e_tiles[s] = e_t
                # fillers first (one per step when eligible): av-rotation
                # parity requires filler tiles to precede a block's av tile
                # allocated in the same step
                if fill_ptr < len(fillers) and fillers[fill_ptr][1] <= s:
                    fillers[fill_ptr][0]()
                    fill_ptr += 1
                # trailing AV stream
                while av_ptr < n_steps and av_ptr <= s - LAGS[av_ptr // JT]:
                    emit_av(av_ptr)
                    av_ptr += 1

            # flush
            while fill_ptr < len(fillers):
                fillers[fill_ptr][0]()
                fill_ptr += 1
            while av_ptr < n_steps:
                emit_av(av_ptr)
                av_ptr += 1
            for pair in range(NOUT // 2):
                out_pair(1, pair)

    nc.finalize()
    return nc


def make_in_maps(query, key, value, Wq, bq, Wk, bk, Wv, bv, Wo, bo,
                 dtype="bf16"):
    """Shard full inputs into the 8 per-core input dicts."""
    import ml_dtypes
    f = lambda a: np.asarray(a, dtype=np.float32)
    query, key, value = f(query), f(key), f(value)
    Wq, Wk, Wv, Wo = f(Wq), f(Wk), f(Wv), f(Wo)
    bq, bk, bv, bo = f(bq), f(bk), f(bv), f(bo)
    cvt = lambda a: np.ascontiguousarray(a.astype(ml_dtypes.bfloat16))

    def warr(W, sl):
        # [D, DH] slice -> [P, KT, DH] with w[p, kt, n] = W[kt*P+p, sl][n]
        return cvt(W[:, sl].reshape(KT, P, DH).transpose(1, 0, 2))

    in_maps = []
    for core in range(N_CORES):
        b, g = core // GROUPS, core % GROUPS
        sl = slice(g * DH, (g + 1) * DH)
        vT = value[b].T  # [D, S]
        v_arr = vT[:, :].reshape(KT, P, JP, 2 * P).transpose(1, 2, 0, 3)
        in_maps.append({
            "qT": cvt(query[b].T),
            "kTd": cvt(key[b].T),
            "v_arr": cvt(np.ascontiguousarray(v_arr)),
            "wq": warr(Wq, sl),
            "wk": warr(Wk, sl),
            "wv": warr(Wv, sl),
            "wo": cvt(Wo[sl, :].reshape(HC, P, D).transpose(1, 0, 2)),
            "bq": np.ascontiguousarray(bq[sl].reshape(HC, P).T),
            "bk": np.ascontiguousarray(bk[sl].reshape(HC, P).T),
            "bvb": np.ascontiguousarray(
                np.broadcast_to(bv[sl].reshape(H_CORE, DK)[None],
                                (P, H_CORE, DK))),
            "bo": (np.ascontiguousarray(bo.reshape(NOUT, P).T)
                   if g == 0 else np.zeros((P, NOUT), np.float32)),
        })
    return in_maps


# test hooks (ignored by the harness)
TRACE = False
LAST_RESULT = None
DTYPE = "bf16"
_NC_CACHE = {}


def kernel(query, key, value, Wq, bq, Wk, bk, Wv, bv, Wo, bo):
    global LAST_RESULT
    from concourse.bass_utils import run_bass_kernel_spmd

    if DTYPE not in _NC_CACHE:
        _NC_CACHE[DTYPE] = build_nc(dtype=DTYPE)
    nc = _NC_CACHE[DTYPE]

    in_maps = make_in_maps(query, key, value, Wq, bq, Wk, bk, Wv, bv, Wo, bo,
                           dtype=DTYPE)
    kwargs = {}
    if TRACE:
        kwargs = dict(trace=True, trace_cores=[0])
    res = run_bass_kernel_spmd(nc, in_maps, core_ids=list(range(N_CORES)),
                               **kwargs)
    LAST_RESULT = res

    out = np.zeros((B, S, D), np.float32)
    for core in range(N_CORES):
        b = core // GROUPS
        out[b] += res.results[core]["outT"].astype(np.float32).T
    return out


# revision 17
# speedup vs baseline: 1.4309x; 1.0222x over previous
"""Multi-head attention, tensor-parallel across 8 Trainium2 NeuronCores.

Sharding: core = (batch b, head-group g) with b in {0,1}, g in {0..3}.
Each core computes 4 heads (a 256-wide slice of the head dimension) for one
batch element.

Schedule (single software-pipelined stream, ACT-bound steady state):
  - K projection runs kt-outer so PE tracks the kT input DMA arrival;
    8 psum chunk accumulators live in the sc/av psum slots before the
    attention rotation starts.
  - Q projection (first half of S) likewise; attention scores+exp begin
    ~23us in.  The rest of Q, all of V, and the ib0 output projection are
    interleaved into the attention stream as PE fillers.
  - Attention: per step (ibx, h, jt): 2 score matmuls (K=64, base-64) into
    a double-buffered [128,1024] psum tile, one exp on ACT, and AV matmuls
    trailing by a per-block lag (so V production can stay ahead early on).
    AV accumulates [65,1024] per block with an appended ones-column giving
    the softmax denominator Z as row 64.
  - Normalization per block: 1/Z via reciprocal_approx_fast, DRAM
    round-trip broadcast to 64 partitions, single fused psum-multiply.
  - Output projection per ibx: pairs of [128,1024] psum tiles borrowed
    from the score rotation; ib0 during ib1's attention, ib1 in the tail.

Inputs arrive full-size; all sharding is internal; host pre-arranges
weight/value layouts so every device DMA is contiguous per partition.
"""

import numpy as np

# Problem shape (hardcoded per the harness contract).
B, S, D, H = 2, 2048, 1024, 16
DK = D // H              # 64 head dim
N_CORES = 8
GROUPS = N_CORES // B    # 4 head-groups
DH = D // GROUPS         # 256 head-dims per core (4 heads)
H_CORE = DH // DK        # 4 heads per core
SCALE = 1.0 / float(np.sqrt(DK))

P = 128                  # SBUF/PSUM partitions
SC = 512                 # matmul moving-dim chunk (one PSUM bank of fp32)
IB = 1024                # i-block (exp granule, AV accumulator width)

KT = D // P              # 8 contraction tiles for projections
NSC = S // SC            # 4 s chunks
HC = DH // P             # 2 head-dim chunks per core
HPC = P // DK            # 2 heads per chunk
JT = S // P              # 16 j tiles
JP = S // (2 * P)        # 8 j pairs (V column-tile granularity)
NIB = S // IB            # 2 i blocks
NOUT = D // P            # 8 output row chunks

# per-block AV lag (steps the AV stream trails the score/exp stream);
# large early so V-projection fillers stay ahead of AV consumption.
LAGS = [17, 14, 11, 8, 6, 4, 4, 4]
E_BUFS = 20


def build_nc(dtype="bf16"):
    """Build the per-core Bass module (same NEFF for all 8 cores)."""
    import concourse.bacc as bacc
    import concourse.mybir as mybir
    import concourse.tile as tile

    f32 = mybir.dt.float32
    bf16 = mybir.dt.bfloat16
    Exp = mybir.ActivationFunctionType.Exp
    cdt = bf16

    nc = bacc.Bacc("TRN2", target_bir_lowering=False, debug=False)

    qT = nc.dram_tensor("qT", [D, S], cdt, kind="ExternalInput")
    kTd = nc.dram_tensor("kTd", [D, S], cdt, kind="ExternalInput")
    v_arr = nc.dram_tensor("v_arr", [P, JP, KT, 2 * P], cdt, kind="ExternalInput")
    wq = nc.dram_tensor("wq", [P, KT, DH], cdt, kind="ExternalInput")
    wk = nc.dram_tensor("wk", [P, KT, DH], cdt, kind="ExternalInput")
    wv = nc.dram_tensor("wv", [P, KT, DH], cdt, kind="ExternalInput")
    wo = nc.dram_tensor("wo", [P, HC, D], cdt, kind="ExternalInput")
    bq = nc.dram_tensor("bq", [P, HC], f32, kind="ExternalInput")
    bk = nc.dram_tensor("bk", [P, HC], f32, kind="ExternalInput")
    bvb = nc.dram_tensor("bvb", [P, H_CORE, DK], f32, kind="ExternalInput")
    bo = nc.dram_tensor("bo", [P, NOUT], f32, kind="ExternalInput")
    outT = nc.dram_tensor("outT", [D, S], cdt, kind="ExternalOutput")

    with tile.TileContext(nc) as tc:
        with (
            tc.tile_pool(name="const", bufs=1) as cpool,
            tc.tile_pool(name="pers", bufs=1) as pers,
            tc.tile_pool(name="stream", bufs=1) as stream,
            tc.tile_pool(name="psum", bufs=1, space="PSUM") as psum,
            tc.tile_pool(name="dscratch", bufs=1, space="DRAM") as dscratch,
        ):
            # ---- constants (host pre-arranged; contiguous per partition) ----
            wk_sb = cpool.tile([P, KT, DH], cdt, name="wk_sb")
            wq_sb = cpool.tile([P, KT, DH], cdt, name="wq_sb")
            wv_sb = cpool.tile([P, KT, DH], cdt, name="wv_sb")
            wo_sb = cpool.tile([P, HC, D], cdt, name="wo_sb")
            bq_sb = cpool.tile([P, HC], f32, name="bq_sb")
            bk_sb = cpool.tile([P, HC], f32, name="bk_sb")
            bvb_sb = cpool.tile([P, H_CORE, DK], f32, name="bvb_sb")
            bo_sb = cpool.tile([P, NOUT], f32, name="bo_sb")

            # ---- persistent activations ----
            # Q^T/K^T per head on partitions 64-127 (base-64 K=64 matmuls
            # sustain full rate).
            qt_h = [pers.tile([P, S], cdt, name=f"qth{h}") for h in range(H_CORE)]
            kt_h = [pers.tile([P, S], cdt, name=f"kth{h}") for h in range(H_CORE)]
            v_c = [pers.tile([P, JT, HPC, DK + 1], cdt, name=f"v{c}")
                   for c in range(HC)]
            on_c = [pers.tile([P, S], cdt, name=f"on{c}") for c in range(HC)]

            def sc_tile(name):
                return psum.tile([P, IB], f32, tag="sc", bufs=2, name=name)

            def av_tile(name):
                return psum.tile([P, IB], f32, tag="av", bufs=2, name=name)

            # ---- DMA issue: weights + inputs, in consumption order ----
            # kT + qT01 stream on SP at full bandwidth; vin reuses the qin
            # slots and qin23 the kin slots, so their transfers are held (WAR)
            # until the k-/q01-projection matmuls consume the first users --
            # a free throttle that keeps the early loads uncontended.  vin /
            # qin23 issue from gpsimd so their slot-waits never block SP.
            for c in range(HC):
                nc.gpsimd.memset(v_c[c][:, :, :, DK:DK + 1], 1.0)
            nc.sync.dma_start(wk_sb[:], wk[:, :, :])
            nc.sync.dma_start(bk_sb[:], bk[:, :])
            kin = []
            for kt in range(KT):
                t = stream.tile([P, S], cdt, tag="kin", bufs=KT,
                                name=f"kin{kt}")
                nc.sync.dma_start(t[:], kTd[kt * P:(kt + 1) * P, :])
                kin.append(t)
            nc.sync.dma_start(wq_sb[:], wq[:, :, :])
            nc.sync.dma_start(bq_sb[:], bq[:, :])
            qin = []
            for kt in range(KT):
                t = stream.tile([P, S // 2], cdt, tag="qin", bufs=KT,
                                name=f"qin01_{kt}")
                nc.sync.dma_start(t[:], qT[kt * P:(kt + 1) * P, 0:S // 2])
                qin.append(t)
            nc.sync.dma_start(wv_sb[:], wv[:, :, :])
            nc.sync.dma_start(bvb_sb[:], bvb[:, :, :])
            nc.sync.dma_start(wo_sb[:], wo[:, :, :])
            nc.sync.dma_start(bo_sb[:], bo[:, :])
            vin = []
            for jp in range(JP):
                t = stream.tile([P, KT, 2 * P], cdt, tag="qin", bufs=KT,
                                name=f"vin{jp}")
                nc.gpsimd.dma_start(t[:], v_arr[:, jp, :, :])
                vin.append(t)
            qin23 = []
            for kt in range(KT):
                t = stream.tile([P, S // 2], cdt, tag="qin", bufs=KT,
                                name=f"qin23_{kt}")
                nc.gpsimd.dma_start(t[:], qT[kt * P:(kt + 1) * P, S // 2:S])
                qin23.append(t)

            # PE p-state warmup: garbage matmuls on already-loaded weights so
            # the tensor engine reaches max clock before the k-projection
            # (ramp requires ~3us of continuous execution)
            warm = psum.tile([P, IB], f32, tag="sc", bufs=2, name="warm")
            for i in range(24):
                nc.tensor.matmul(
                    warm[:, (i % 2) * SC:(i % 2 + 1) * SC],
                    lhsT=wk_sb[:, 0, 0:P], rhs=wk_sb[:, 0:2, :],
                    start=True, stop=True)

            # ---- K projection: kt-outer, 8 chunk accumulators ----
            # chunk (c, si) -> tile (c*2 + si//2), half si%2
            kboot = [sc_tile("kb0"), sc_tile("kb1"), av_tile("kb2"),
                     av_tile("kb3")]
            for kt in range(KT):
                for c in range(HC):
                    for si in range(NSC):
                        dst = kboot[c * 2 + si // 2]
                        half = si % 2
                        nc.tensor.matmul(
                            dst[:, half * SC:(half + 1) * SC],
                            lhsT=wk_sb[:, kt, c * P:(c + 1) * P],
                            rhs=kin[kt][:, si * SC:(si + 1) * SC],
                            start=(kt == 0), stop=(kt == KT - 1))
            with tc.high_priority():
                for c in range(HC):
                    stg = stream.tile([P, S], cdt, tag="stg", bufs=2,
                                      name=f"stgk{c}")
                    for half in range(2):
                        nc.vector.tensor_scalar_add(
                            stg[:, half * IB:(half + 1) * IB],
                            kboot[c * 2 + half][:, :], bk_sb[:, c:c + 1])
                    nc.sync.dma_start(kt_h[c * HPC][DK:P, :], stg[0:DK, :])
                    nc.sync.dma_start(kt_h[c * HPC + 1][DK:P, :],
                                      stg[DK:P, :])

            # ---- Q projection, si 0-1 (i cols 0:1024) ----
            qboot = [sc_tile("qb0"), av_tile("qb1")]  # tile c, halves si
            for kt in range(KT):
                for c in range(HC):
                    for si in range(2):
                        nc.tensor.matmul(
                            qboot[c][:, si * SC:(si + 1) * SC],
                            lhsT=wq_sb[:, kt, c * P:(c + 1) * P],
                            rhs=qin[kt][:, si * SC:(si + 1) * SC],
                            start=(kt == 0), stop=(kt == KT - 1))
            with tc.high_priority():
                for c in range(HC):
                    stg = stream.tile([P, S], cdt, tag="stg", bufs=2,
                                      name=f"stgq01_{c}")
                    nc.vector.tensor_scalar_add(stg[:, 0:IB], qboot[c][:, :],
                                                bq_sb[:, c:c + 1])
                    nc.sync.dma_start(qt_h[c * HPC][DK:P, 0:IB],
                                      stg[0:DK, 0:IB])
                    nc.sync.dma_start(qt_h[c * HPC + 1][DK:P, 0:IB],
                                      stg[DK:P, 0:IB])

            # ---- filler emitters (run between attention steps) ----
            def v_chain(jt):
                """One V j-tile: 8 matmuls accumulating [P,256] in its own
                psum bank (a start-flag matmul zeroes the whole 2KB bank, so
                each chain gets a full [P,SC] half), plus bias-add drains."""
                q = jt % 2
                if q == 0:
                    v_chain.cur = av_tile(f"vps{jt // 2}")
                ps = v_chain.cur[:, q * SC:q * SC + 2 * P]
                for kt in range(KT):
                    nc.tensor.matmul(
                        ps,
                        lhsT=vin[jt // 2][:, kt, (jt % 2) * P:(jt % 2 + 1) * P],
                        rhs=wv_sb[:, kt, :],
                        start=(kt == 0), stop=(kt == KT - 1))
                for c in range(HC):
                    nc.vector.tensor_add(
                        v_c[c][:, jt, :, 0:DK],
                        ps.rearrange("p (h d) -> p h d", d=DK)[
                            :, c * HPC:(c + 1) * HPC, :],
                        bvb_sb[:, c * HPC:(c + 1) * HPC, :])

            def q23_item(c):
                """Q projection chunk (c, si 2-3) in one av-slot tile."""
                ps = av_tile(f"q23_{c}")
                for kt in range(KT):
                    for si in range(2):
                        nc.tensor.matmul(
                            ps[:, si * SC:(si + 1) * SC],
                            lhsT=wq_sb[:, kt, c * P:(c + 1) * P],
                            rhs=qin23[kt][:, si * SC:(si + 1) * SC],
                            start=(kt == 0), stop=(kt == KT - 1))
                stg = stream.tile([P, S], cdt, tag="stg", bufs=2,
                                  name=f"stgq23_{c}")
                nc.vector.tensor_scalar_add(stg[:, 0:IB], ps[:, :],
                                            bq_sb[:, c:c + 1])
                nc.sync.dma_start(qt_h[c * HPC][DK:P, IB:S], stg[0:DK, 0:IB])
                nc.sync.dma_start(qt_h[c * HPC + 1][DK:P, IB:S],
                                  stg[DK:P, 0:IB])

            def out_pair(ibx, pair, mk=None):
                """Output projection for rows [256*pair, 256*(pair+1)) of
                outT, i-cols of ibx: two psum tiles from mk (paired to keep
                rotation parity for long-lived av users)."""
                for sub in range(2):
                    n = pair * 2 + sub
                    ps = (mk or av_tile)(f"ops{ibx}_{n}")
                    for ic in range(2):
                        for c in range(HC):
                            nc.tensor.matmul(
                                ps[:, ic * SC:(ic + 1) * SC],
                                lhsT=wo_sb[:, c, n * P:(n + 1) * P],
                                rhs=on_c[c][:, ibx * IB + ic * SC:
                                            ibx * IB + (ic + 1) * SC],
                                start=(c == 0), stop=(c == HC - 1))
                    o_sb = stream.tile([P, IB], cdt, tag="osb", bufs=3,
                                       name=f"osb{ibx}_{n}")
                    nc.vector.tensor_scalar_add(o_sb[:, :], ps[:, :],
                                                bo_sb[:, n:n + 1])
                    nc.sync.dma_start(
                        outT[n * P:(n + 1) * P, ibx * IB:(ibx + 1) * IB],
                        o_sb[:, :])

            fillers = []
            for jt in range(JT):
                fillers.append((lambda jt=jt: v_chain(jt), 0))
            for c in range(HC):
                # both q23 tiles must enter the av rotation before blk0's
                # long-lived tile so the 2-slot parity keeps every blkN+1
                # waiting on a short-lived user, not on blkN
                fillers.append((lambda c=c: q23_item(c), LAGS[0] - 1))

            # ---- attention stream ----
            steps = [(ibx, h, jt)
                     for ibx in range(NIB)
                     for h in range(H_CORE)
                     for jt in range(JT)]
            n_steps = len(steps)
            e_tiles = [None] * n_steps
            blk_av = {}

            def emit_av(s):
                ibx, h, jt = steps[s]
                blk = s // JT
                hc, hh = h // HPC, h % HPC
                if jt == 0:
                    blk_av[blk] = av_tile(f"avb{blk}")
                av = blk_av[blk]
                e_t = e_tiles[s]
                for ic in range(2):
                    nc.tensor.matmul(
                        av[0:DK + 1, ic * SC:(ic + 1) * SC],
                        lhsT=v_c[hc][:, jt, hh, :],
                        rhs=e_t[:, ic * SC:(ic + 1) * SC],
                        start=(jt == 0), stop=(jt == JT - 1))
                e_tiles[s] = None
                if jt == JT - 1:
                    emit_norm(blk, av)
                    # ib0 output projection rides the av rotation right after
                    # a block's copy frees its slot (pairs keep parity)
                    if blk == 4:
                        out_pair(0, 0)
                        out_pair(0, 1)
                    elif blk == 5:
                        out_pair(0, 2)
                        out_pair(0, 3)

            def emit_norm(blk, av):
                """Copy [O;Z] off psum (frees the av slot fast), spread the
                Z row over 64 partitions via a DRAM round trip so the
                microcoded DVE reciprocal runs on free-size 16 instead of
                1024, broadcast 1/Z back, and apply in one fused multiply."""
                ibx = blk // H_CORE
                h = blk % H_CORE
                hc, hh = h // HPC, h % HPC
                avs = stream.tile([P, IB], f32, tag="avsb", bufs=2,
                                  name=f"avsb{blk}")
                nc.vector.tensor_copy(avs[0:DK + 1, :], av[0:DK + 1, :])
                z_d = dscratch.tile([1, IB], f32, tag="zd", bufs=2,
                                    name=f"zd{blk}")
                nc.sync.dma_start(z_d[:], avs[DK:DK + 1, :])
                zq = stream.tile([P, IB // DK], f32, tag="zq", bufs=2,
                                 name=f"zq{blk}")
                nc.sync.dma_start(
                    zq[0:DK, :],
                    z_d[:, :].rearrange("a (p f) -> (a p) f", p=DK))
                zqr = stream.tile([P, IB // DK], f32, tag="zqr", bufs=2,
                                  name=f"zqr{blk}")
                nc.vector.reciprocal(zqr[0:DK, :], zq[0:DK, :])
                rz_d = dscratch.tile([1, IB], f32, tag="rzd", bufs=2,
                                     name=f"rzd{blk}")
                nc.sync.dma_start(
                    rz_d[:, :].rearrange("a (p f) -> (a p) f", p=DK),
                    zqr[0:DK, :])
                rzb = stream.tile([P, IB], f32, tag="rzb", bufs=2,
                                  name=f"rzb{blk}")
                nc.sync.dma_start(rzb[0:DK, :],
                                  rz_d[:, :].to_broadcast((DK, IB)))
                ot = stream.tile([P, IB], cdt, tag="ot", bufs=2,
                                 name=f"ot{blk}")
                nc.vector.tensor_mul(ot[0:DK, :], avs[0:DK, :], rzb[0:DK, :])
                nc.sync.dma_start(
                    on_c[hc][hh * DK:(hh + 1) * DK,
                             ibx * IB:(ibx + 1) * IB],
                    ot[0:DK, :])

            av_ptr = 0
            fill_ptr = 0
            for s in range(n_steps):
                ibx, h, jt = steps[s]
                blk = s // JT
                sc = sc_tile(f"sc{s}")
                for ic in range(2):
                    nc.tensor.matmul(
                        sc[:, ic * SC:(ic + 1) * SC],
                        lhsT=kt_h[h][DK:P, jt * P:(jt + 1) * P],
                        rhs=qt_h[h][DK:P, ibx * IB + ic * SC:
                                    ibx * IB + (ic + 1) * SC],
                        start=True, stop=True)
                e_t = stream.tile([P, IB], cdt, tag="e", bufs=E_BUFS,
                                  name=f"e{s}")
                nc.scalar.activation(e_t[:], sc[:], Exp, bias=0.0,
                                     scale=SCALE)
                e_tiles[s] = e_t
                # fillers first (one per step when eligible): av-rotation
                # parity requires filler tiles to precede a block's av tile
                # allocated in the same step
                if fill_ptr < len(fillers) and fillers[fill_ptr][1] <= s:
                    fillers[fill_ptr][0]()
                    fill_ptr += 1
                # trailing AV stream
                while av_ptr < n_steps and av_ptr <= s - LAGS[av_ptr // JT]:
                    emit_av(av_ptr)
                    av_ptr += 1

            # flush
            while fill_ptr < len(fillers):
                fillers[fill_ptr][0]()
                fill_ptr += 1
            while av_ptr < n_steps:
                emit_av(av_ptr)
                av_ptr += 1
            # tail: both psum tags are free now — alternate for deeper overlap
            for pair in range(NOUT // 2):
                out_pair(1, pair, mk=(sc_tile if pair % 2 else av_tile))

    nc.finalize()
    return nc


def make_in_maps(query, key, value, Wq, bq, Wk, bk, Wv, bv, Wo, bo,
                 dtype="bf16"):
    """Shard full inputs into the 8 per-core input dicts."""
    import ml_dtypes
    f = lambda a: np.asarray(a, dtype=np.float32)
    query, key, value = f(query), f(key), f(value)
    Wq, Wk, Wv, Wo = f(Wq), f(Wk), f(Wv), f(Wo)
    bq, bk, bv, bo = f(bq), f(bk), f(bv), f(bo)
    cvt = lambda a: np.ascontiguousarray(a.astype(ml_dtypes.bfloat16))

    def warr(W, sl):
        # [D, DH] slice -> [P, KT, DH] with w[p, kt, n] = W[kt*P+p, sl][n]
        return cvt(W[:, sl].reshape(KT, P, DH).transpose(1, 0, 2))

    in_maps = []
    for core in range(N_CORES):
        b, g = core // GROUPS, core % GROUPS
        sl = slice(g * DH, (g + 1) * DH)
        vT = value[b].T  # [D, S]
        v_arr = vT[:, :].reshape(KT, P, JP, 2 * P).transpose(1, 2, 0, 3)
        in_maps.append({
            "qT": cvt(query[b].T),
            "kTd": cvt(key[b].T),
            "v_arr": cvt(np.ascontiguousarray(v_arr)),
            "wq": warr(Wq, sl),
            "wk": warr(Wk, sl),
            "wv": warr(Wv, sl),
            "wo": cvt(Wo[sl, :].reshape(HC, P, D).transpose(1, 0, 2)),
            "bq": np.ascontiguousarray(bq[sl].reshape(HC, P).T),
            "bk": np.ascontiguousarray(bk[sl].reshape(HC, P).T),
            "bvb": np.ascontiguousarray(
                np.broadcast_to(bv[sl].reshape(H_CORE, DK)[None],
                                (P, H_CORE, DK))),
            "bo": (np.ascontiguousarray(bo.reshape(NOUT, P).T)
                   if g == 0 else np.zeros((P, NOUT), np.float32)),
        })
    return in_maps


# test hooks (ignored by the harness)
TRACE = False
LAST_RESULT = None
DTYPE = "bf16"
_NC_CACHE = {}


def kernel(query, key, value, Wq, bq, Wk, bk, Wv, bv, Wo, bo):
    global LAST_RESULT
    from concourse.bass_utils import run_bass_kernel_spmd

    if DTYPE not in _NC_CACHE:
        _NC_CACHE[DTYPE] = build_nc(dtype=DTYPE)
    nc = _NC_CACHE[DTYPE]

    in_maps = make_in_maps(query, key, value, Wq, bq, Wk, bk, Wv, bv, Wo, bo,
                           dtype=DTYPE)
    kwargs = {}
    if TRACE:
        kwargs = dict(trace=True, trace_cores=[0])
    res = run_bass_kernel_spmd(nc, in_maps, core_ids=list(range(N_CORES)),
                               **kwargs)
    LAST_RESULT = res

    out = np.zeros((B, S, D), np.float32)
    for core in range(N_CORES):
        b = core // GROUPS
        out[b] += res.results[core]["outT"].astype(np.float32).T
    return out


# revision 21
# speedup vs baseline: 1.4577x; 1.0187x over previous
"""Multi-head attention, tensor-parallel across 8 Trainium2 NeuronCores.

Sharding: core = (batch b, head-group g) with b in {0,1}, g in {0..3}.
Each core computes 4 heads (a 256-wide slice of the head dimension) for one
batch element.

Schedule (single software-pipelined stream, ACT-bound steady state):
  - K projection runs kt-outer so PE tracks the kT input DMA arrival;
    8 psum chunk accumulators live in the sc/av psum slots before the
    attention rotation starts.
  - Q projection (first half of S) likewise; attention scores+exp begin
    ~23us in.  The rest of Q, all of V, and the ib0 output projection are
    interleaved into the attention stream as PE fillers.
  - Attention: per step (ibx, h, jt): 2 score matmuls (K=64, base-64) into
    a double-buffered [128,1024] psum tile, one exp on ACT, and AV matmuls
    trailing by a per-block lag (so V production can stay ahead early on).
    AV accumulates [65,1024] per block with an appended ones-column giving
    the softmax denominator Z as row 64.
  - Normalization per block: 1/Z via reciprocal_approx_fast, DRAM
    round-trip broadcast to 64 partitions, single fused psum-multiply.
  - Output projection per ibx: pairs of [128,1024] psum tiles borrowed
    from the score rotation; ib0 during ib1's attention, ib1 in the tail.

Inputs arrive full-size; all sharding is internal; host pre-arranges
weight/value layouts so every device DMA is contiguous per partition.
"""

import numpy as np

# Problem shape (hardcoded per the harness contract).
B, S, D, H = 2, 2048, 1024, 16
DK = D // H              # 64 head dim
N_CORES = 8
GROUPS = N_CORES // B    # 4 head-groups
DH = D // GROUPS         # 256 head-dims per core (4 heads)
H_CORE = DH // DK        # 4 heads per core
SCALE = 1.0 / float(np.sqrt(DK))

P = 128                  # SBUF/PSUM partitions
SC = 512                 # matmul moving-dim chunk (one PSUM bank of fp32)
IB = 1024                # i-block (exp granule, AV accumulator width)

KT = D // P              # 8 contraction tiles for projections
NSC = S // SC            # 4 s chunks
HC = DH // P             # 2 head-dim chunks per core
HPC = P // DK            # 2 heads per chunk
JT = S // P              # 16 j tiles
JP = S // (2 * P)        # 8 j pairs (V column-tile granularity)
NIB = S // IB            # 2 i blocks
NOUT = D // P            # 8 output row chunks

# per-block AV lag (steps the AV stream trails the score/exp stream);
# large early so V-projection fillers stay ahead of AV consumption; small
# for the last block so the tail flush is short.
LAGS = [18, 14, 11, 8, 6, 4, 4, 2]
E_BUFS = 20


def build_nc(dtype="bf16"):
    """Build the per-core Bass module (same NEFF for all 8 cores)."""
    import concourse.bacc as bacc
    import concourse.mybir as mybir
    import concourse.tile as tile

    f32 = mybir.dt.float32
    bf16 = mybir.dt.bfloat16
    Exp = mybir.ActivationFunctionType.Exp
    cdt = bf16

    nc = bacc.Bacc("TRN2", target_bir_lowering=False, debug=False)

    qT = nc.dram_tensor("qT", [D, S], cdt, kind="ExternalInput")
    kTd = nc.dram_tensor("kTd", [D, S], cdt, kind="ExternalInput")
    v_arr = nc.dram_tensor("v_arr", [P, JP, KT, 2 * P], cdt, kind="ExternalInput")
    wq = nc.dram_tensor("wq", [P, KT, DH], cdt, kind="ExternalInput")
    wk = nc.dram_tensor("wk", [P, KT, DH], cdt, kind="ExternalInput")
    wv = nc.dram_tensor("wv", [P, KT, DH], cdt, kind="ExternalInput")
    wo = nc.dram_tensor("wo", [P, HC, D], cdt, kind="ExternalInput")
    bq = nc.dram_tensor("bq", [P, HC], f32, kind="ExternalInput")
    bk = nc.dram_tensor("bk", [P, HC], f32, kind="ExternalInput")
    bvb = nc.dram_tensor("bvb", [P, H_CORE, DK], f32, kind="ExternalInput")
    bo = nc.dram_tensor("bo", [P, NOUT], f32, kind="ExternalInput")
    outT = nc.dram_tensor("outT", [D, S], cdt, kind="ExternalOutput")

    with tile.TileContext(nc) as tc:
        with (
            tc.tile_pool(name="const", bufs=1) as cpool,
            tc.tile_pool(name="pers", bufs=1) as pers,
            tc.tile_pool(name="stream", bufs=1) as stream,
            tc.tile_pool(name="psum", bufs=1, space="PSUM") as psum,
            tc.tile_pool(name="dscratch", bufs=1, space="DRAM") as dscratch,
        ):
            # ---- constants (host pre-arranged; contiguous per partition) ----
            wk_sb = cpool.tile([P, KT, DH], cdt, name="wk_sb")
            wq_sb = cpool.tile([P, KT, DH], cdt, name="wq_sb")
            wv_sb = cpool.tile([P, KT, DH], cdt, name="wv_sb")
            wo_sb = cpool.tile([P, HC, D], cdt, name="wo_sb")
            bq_sb = cpool.tile([P, HC], f32, name="bq_sb")
            bk_sb = cpool.tile([P, HC], f32, name="bk_sb")
            bvb_sb = cpool.tile([P, H_CORE, DK], f32, name="bvb_sb")
            bo_sb = cpool.tile([P, NOUT], f32, name="bo_sb")

            # ---- persistent activations ----
            # Q^T/K^T per head on partitions 64-127 (base-64 K=64 matmuls
            # sustain full rate).
            qt_h = [pers.tile([P, S], cdt, name=f"qth{h}") for h in range(H_CORE)]
            kt_h = [pers.tile([P, S], cdt, name=f"kth{h}") for h in range(H_CORE)]
            v_c = [pers.tile([P, JT, HPC, DK + 1], cdt, name=f"v{c}")
                   for c in range(HC)]
            on_c = [pers.tile([P, S], cdt, name=f"on{c}") for c in range(HC)]

            def sc_tile(name):
                return psum.tile([P, IB], f32, tag="sc", bufs=2, name=name)

            def av_tile(name):
                return psum.tile([P, IB], f32, tag="av", bufs=2, name=name)

            # ---- DMA issue: weights + inputs, in consumption order ----
            # kT + qT01 stream on SP at full bandwidth; vin reuses the qin
            # slots and qin23 the kin slots, so their transfers are held (WAR)
            # until the k-/q01-projection matmuls consume the first users --
            # a free throttle that keeps the early loads uncontended.  vin /
            # qin23 issue from gpsimd so their slot-waits never block SP.
            for c in range(HC):
                nc.gpsimd.memset(v_c[c][:, :, :, DK:DK + 1], 1.0)
            nc.sync.dma_start(wk_sb[:], wk[:, :, :])
            nc.sync.dma_start(bk_sb[:], bk[:, :])
            kin = []
            for kt in range(KT):
                t = stream.tile([P, S], cdt, tag="kin", bufs=KT,
                                name=f"kin{kt}")
                nc.sync.dma_start(t[:], kTd[kt * P:(kt + 1) * P, :])
                kin.append(t)
            nc.sync.dma_start(wq_sb[:], wq[:, :, :])
            nc.sync.dma_start(bq_sb[:], bq[:, :])
            qin = []
            for kt in range(KT):
                t = stream.tile([P, S // 2], cdt, tag="qin", bufs=KT,
                                name=f"qin01_{kt}")
                nc.sync.dma_start(t[:], qT[kt * P:(kt + 1) * P, 0:S // 2])
                qin.append(t)
            nc.sync.dma_start(wv_sb[:], wv[:, :, :])
            nc.sync.dma_start(bvb_sb[:], bvb[:, :, :])
            nc.sync.dma_start(wo_sb[:], wo[:, :, :])
            nc.sync.dma_start(bo_sb[:], bo[:, :])
            vin = []
            for jp in range(JP):
                t = stream.tile([P, KT, 2 * P], cdt, tag="qin", bufs=KT,
                                name=f"vin{jp}")
                nc.gpsimd.dma_start(t[:], v_arr[:, jp, :, :])
                vin.append(t)
            qin23 = []
            for kt in range(KT):
                t = stream.tile([P, S // 2], cdt, tag="qin", bufs=KT,
                                name=f"qin23_{kt}")
                nc.gpsimd.dma_start(t[:], qT[kt * P:(kt + 1) * P, S // 2:S])
                qin23.append(t)

            # PE p-state warmup: garbage matmuls on already-loaded weights so
            # the tensor engine reaches max clock before the k-projection
            # (ramp requires ~3us of continuous execution); short enough not
            # to delay the first k matmuls once kin[0] lands
            warm = psum.tile([P, IB], f32, tag="sc", bufs=2, name="warm")
            for i in range(10):
                nc.tensor.matmul(
                    warm[:, (i % 2) * SC:(i % 2 + 1) * SC],
                    lhsT=wk_sb[:, 0, 0:P], rhs=wk_sb[:, 0:2, :],
                    start=True, stop=True)

            # ---- K + Q(cols 0:1024) projections, chunk c per phase ----
            # kt-outer so PE tracks input DMA arrival; phase 1 covers heads
            # 0/1 (chunk 0) for both K and Q so the first scores can start
            # right after the last input tile lands; phase 2 (heads 2/3)
            # re-reads the resident input tiles kt-ascending, releasing the
            # qin/kin slots for the throttled vin/qin23 loads.
            def kq_phase(c, kbt, qbt):
                for kt in range(KT):
                    for si in range(NSC):
                        dst = kbt[si // 2]
                        nc.tensor.matmul(
                            dst[:, (si % 2) * SC:(si % 2 + 1) * SC],
                            lhsT=wk_sb[:, kt, c * P:(c + 1) * P],
                            rhs=kin[kt][:, si * SC:(si + 1) * SC],
                            start=(kt == 0), stop=(kt == KT - 1))
                    for si in range(2):
                        nc.tensor.matmul(
                            qbt[:, si * SC:(si + 1) * SC],
                            lhsT=wq_sb[:, kt, c * P:(c + 1) * P],
                            rhs=qin[kt][:, si * SC:(si + 1) * SC],
                            start=(kt == 0), stop=(kt == KT - 1))
                with tc.high_priority():
                    stg = stream.tile([P, S], cdt, tag="stg", bufs=2,
                                      name=f"stgq{c}")
                    nc.vector.tensor_scalar_add(stg[:, 0:IB], qbt[:, :],
                                                bq_sb[:, c:c + 1])
                    nc.sync.dma_start(qt_h[c * HPC][DK:P, 0:IB],
                                      stg[0:DK, 0:IB])
                    nc.sync.dma_start(qt_h[c * HPC + 1][DK:P, 0:IB],
                                      stg[DK:P, 0:IB])
                    stgk = stream.tile([P, S], cdt, tag="stg", bufs=2,
                                       name=f"stgk{c}")
                    for half in range(2):
                        nc.vector.tensor_scalar_add(
                            stgk[:, half * IB:(half + 1) * IB],
                            kbt[half][:, :], bk_sb[:, c:c + 1])
                    nc.sync.dma_start(kt_h[c * HPC][DK:P, :], stgk[0:DK, :])
                    nc.sync.dma_start(kt_h[c * HPC + 1][DK:P, :],
                                      stgk[DK:P, :])

            kq_phase(0, [sc_tile("kb0"), sc_tile("kb1")], av_tile("qb0"))
            kq_phase(1, [av_tile("kb2"), sc_tile("kb3")], sc_tile("qb1"))

            # ---- filler emitters (run between attention steps) ----
            def v_chain(jt):
                """One V j-tile: 8 matmuls accumulating [P,256] in its own
                psum bank (a start-flag matmul zeroes the whole 2KB bank, so
                each chain gets a full [P,SC] half), plus bias-add drains."""
                q = jt % 2
                if q == 0:
                    v_chain.cur = av_tile(f"vps{jt // 2}")
                ps = v_chain.cur[:, q * SC:q * SC + 2 * P]
                for kt in range(KT):
                    nc.tensor.matmul(
                        ps,
                        lhsT=vin[jt // 2][:, kt, (jt % 2) * P:(jt % 2 + 1) * P],
                        rhs=wv_sb[:, kt, :],
                        start=(kt == 0), stop=(kt == KT - 1))
                for c in range(HC):
                    nc.vector.tensor_add(
                        v_c[c][:, jt, :, 0:DK],
                        ps.rearrange("p (h d) -> p h d", d=DK)[
                            :, c * HPC:(c + 1) * HPC, :],
                        bvb_sb[:, c * HPC:(c + 1) * HPC, :])

            def q23_item(c):
                """Q projection chunk (c, si 2-3) in one av-slot tile."""
                ps = av_tile(f"q23_{c}")
                for kt in range(KT):
                    for si in range(2):
                        nc.tensor.matmul(
                            ps[:, si * SC:(si + 1) * SC],
                            lhsT=wq_sb[:, kt, c * P:(c + 1) * P],
                            rhs=qin23[kt][:, si * SC:(si + 1) * SC],
                            start=(kt == 0), stop=(kt == KT - 1))
                stg = stream.tile([P, S], cdt, tag="stg", bufs=2,
                                  name=f"stgq23_{c}")
                nc.vector.tensor_scalar_add(stg[:, 0:IB], ps[:, :],
                                            bq_sb[:, c:c + 1])
                nc.sync.dma_start(qt_h[c * HPC][DK:P, IB:S], stg[0:DK, 0:IB])
                nc.sync.dma_start(qt_h[c * HPC + 1][DK:P, IB:S],
                                  stg[DK:P, 0:IB])

            def out_pair(ibx, pair, mk=None):
                """Output projection for rows [256*pair, 256*(pair+1)) of
                outT, i-cols of ibx: two psum tiles from mk (paired to keep
                rotation parity for long-lived av users)."""
                for sub in range(2):
                    n = pair * 2 + sub
                    ps = (mk or av_tile)(f"ops{ibx}_{n}")
                    for ic in range(2):
                        for c in range(HC):
                            nc.tensor.matmul(
                                ps[:, ic * SC:(ic + 1) * SC],
                                lhsT=wo_sb[:, c, n * P:(n + 1) * P],
                                rhs=on_c[c][:, ibx * IB + ic * SC:
                                            ibx * IB + (ic + 1) * SC],
                                start=(c == 0), stop=(c == HC - 1))
                    o_sb = stream.tile([P, IB], cdt, tag="osb", bufs=3,
                                       name=f"osb{ibx}_{n}")
                    nc.vector.tensor_scalar_add(o_sb[:, :], ps[:, :],
                                                bo_sb[:, n:n + 1])
                    nc.sync.dma_start(
                        outT[n * P:(n + 1) * P, ibx * IB:(ibx + 1) * IB],
                        o_sb[:, :])

            fillers = []
            for jt in range(JT):
                fillers.append((lambda jt=jt: v_chain(jt), 0))
            for c in range(HC):
                # both q23 tiles must enter the av rotation before blk0's
                # long-lived tile so the 2-slot parity keeps every blkN+1
                # waiting on a short-lived user, not on blkN
                fillers.append((lambda c=c: q23_item(c), LAGS[0] - 1))

            # ---- attention stream ----
            steps = [(ibx, h, jt)
                     for ibx in range(NIB)
                     for h in range(H_CORE)
                     for jt in range(JT)]
            n_steps = len(steps)
            e_tiles = [None] * n_steps
            blk_av = {}

            def emit_av(s):
                ibx, h, jt = steps[s]
                blk = s // JT
                hc, hh = h // HPC, h % HPC
                if jt == 0:
                    blk_av[blk] = av_tile(f"avb{blk}")
                av = blk_av[blk]
                e_t = e_tiles[s]
                for ic in range(2):
                    nc.tensor.matmul(
                        av[0:DK + 1, ic * SC:(ic + 1) * SC],
                        lhsT=v_c[hc][:, jt, hh, :],
                        rhs=e_t[:, ic * SC:(ic + 1) * SC],
                        start=(jt == 0), stop=(jt == JT - 1))
                e_tiles[s] = None
                if jt == JT - 1:
                    emit_norm(blk, av)
                    # ib0 output projection rides the av rotation right after
                    # a block's copy frees its slot (pairs keep parity)
                    if blk == 4:
                        out_pair(0, 0)
                        out_pair(0, 1)
                    elif blk == 5:
                        out_pair(0, 2)
                        out_pair(0, 3)

            def emit_norm(blk, av):
                """Copy [O;Z] off psum (frees the av slot fast), spread the
                Z row over 64 partitions via a DRAM round trip so the
                microcoded DVE reciprocal runs on free-size 16 instead of
                1024, broadcast 1/Z back, and apply in one fused multiply."""
                ibx = blk // H_CORE
                h = blk % H_CORE
                hc, hh = h // HPC, h % HPC
                avs = stream.tile([P, IB], f32, tag="avsb", bufs=2,
                                  name=f"avsb{blk}")
                nc.vector.tensor_copy(avs[0:DK + 1, :], av[0:DK + 1, :])
                z_d = dscratch.tile([1, IB], f32, tag="zd", bufs=2,
                                    name=f"zd{blk}")
                nc.sync.dma_start(z_d[:], avs[DK:DK + 1, :])
                zq = stream.tile([P, IB // DK], f32, tag="zq", bufs=2,
                                 name=f"zq{blk}")
                nc.sync.dma_start(
                    zq[0:DK, :],
                    z_d[:, :].rearrange("a (p f) -> (a p) f", p=DK))
                zqr = stream.tile([P, IB // DK], f32, tag="zqr", bufs=2,
                                  name=f"zqr{blk}")
                nc.vector.reciprocal(zqr[0:DK, :], zq[0:DK, :])
                rz_d = dscratch.tile([1, IB], f32, tag="rzd", bufs=2,
                                     name=f"rzd{blk}")
                nc.sync.dma_start(
                    rz_d[:, :].rearrange("a (p f) -> (a p) f", p=DK),
                    zqr[0:DK, :])
                rzb = stream.tile([P, IB], f32, tag="rzb", bufs=2,
                                  name=f"rzb{blk}")
                nc.sync.dma_start(rzb[0:DK, :],
                                  rz_d[:, :].to_broadcast((DK, IB)))
                ot = stream.tile([P, IB], cdt, tag="ot", bufs=2,
                                 name=f"ot{blk}")
                nc.vector.tensor_mul(ot[0:DK, :], avs[0:DK, :], rzb[0:DK, :])
                nc.sync.dma_start(
                    on_c[hc][hh * DK:(hh + 1) * DK,
                             ibx * IB:(ibx + 1) * IB],
                    ot[0:DK, :])

            av_ptr = 0
            fill_ptr = 0
            for s in range(n_steps):
                ibx, h, jt = steps[s]
                blk = s // JT
                sc = sc_tile(f"sc{s}")
                for ic in range(2):
                    nc.tensor.matmul(
                        sc[:, ic * SC:(ic + 1) * SC],
                        lhsT=kt_h[h][DK:P, jt * P:(jt + 1) * P],
                        rhs=qt_h[h][DK:P, ibx * IB + ic * SC:
                                    ibx * IB + (ic + 1) * SC],
                        start=True, stop=True)
                e_t = stream.tile([P, IB], cdt, tag="e", bufs=E_BUFS,
                                  name=f"e{s}")
                nc.scalar.activation(e_t[:], sc[:], Exp, bias=0.0,
                                     scale=SCALE)
                e_tiles[s] = e_t
                # fillers first (one per step when eligible): av-rotation
                # parity requires filler tiles to precede a block's av tile
                # allocated in the same step
                if fill_ptr < len(fillers) and fillers[fill_ptr][1] <= s:
                    fillers[fill_ptr][0]()
                    fill_ptr += 1
                # trailing AV stream
                while av_ptr < n_steps and av_ptr <= s - LAGS[av_ptr // JT]:
                    emit_av(av_ptr)
                    av_ptr += 1

            # flush
            while fill_ptr < len(fillers):
                fillers[fill_ptr][0]()
                fill_ptr += 1
            while av_ptr < n_steps:
                emit_av(av_ptr)
                av_ptr += 1
            # keep PE clocked up through the last norm chain so the tail
            # output-projection matmuls run at full rate
            warm2 = sc_tile("warm2")
            for i in range(20):
                nc.tensor.matmul(
                    warm2[:, (i % 2) * SC:(i % 2 + 1) * SC],
                    lhsT=wk_sb[:, 0, 0:P], rhs=wk_sb[:, 0:2, :],
                    start=True, stop=True)
            # tail: both psum tags are free now — alternate for deeper overlap
            for pair in range(NOUT // 2):
                out_pair(1, pair, mk=(sc_tile if pair % 2 else av_tile))

    nc.finalize()
    return nc


def make_in_maps(query, key, value, Wq, bq, Wk, bk, Wv, bv, Wo, bo,
                 dtype="bf16"):
    """Shard full inputs into the 8 per-core input dicts."""
    import ml_dtypes
    f = lambda a: np.asarray(a, dtype=np.float32)
    query, key, value = f(query), f(key), f(value)
    Wq, Wk, Wv, Wo = f(Wq), f(Wk), f(Wv), f(Wo)
    bq, bk, bv, bo = f(bq), f(bk), f(bv), f(bo)
    cvt = lambda a: np.ascontiguousarray(a.astype(ml_dtypes.bfloat16))

    def warr(W, sl):
        # [D, DH] slice -> [P, KT, DH] with w[p, kt, n] = W[kt*P+p, sl][n]
        return cvt(W[:, sl].reshape(KT, P, DH).transpose(1, 0, 2))

    in_maps = []
    for core in range(N_CORES):
        b, g = core // GROUPS, core % GROUPS
        sl = slice(g * DH, (g + 1) * DH)
        vT = value[b].T  # [D, S]
        v_arr = vT[:, :].reshape(KT, P, JP, 2 * P).transpose(1, 2, 0, 3)
        in_maps.append({
            "qT": cvt(query[b].T),
            "kTd": cvt(key[b].T),
            "v_arr": cvt(np.ascontiguousarray(v_arr)),
            "wq": warr(Wq, sl),
            "wk": warr(Wk, sl),
            "wv": warr(Wv, sl),
            "wo": cvt(Wo[sl, :].reshape(HC, P, D).transpose(1, 0, 2)),
            "bq": np.ascontiguousarray(bq[sl].reshape(HC, P).T),
            "bk": np.ascontiguousarray(bk[sl].reshape(HC, P).T),
            "bvb": np.ascontiguousarray(
                np.broadcast_to(bv[sl].reshape(H_CORE, DK)[None],
                                (P, H_CORE, DK))),
            "bo": (np.ascontiguousarray(bo.reshape(NOUT, P).T)
                   if g == 0 else np.zeros((P, NOUT), np.float32)),
        })
    return in_maps


# test hooks (ignored by the harness)
TRACE = False
LAST_RESULT = None
DTYPE = "bf16"
_NC_CACHE = {}


def kernel(query, key, value, Wq, bq, Wk, bk, Wv, bv, Wo, bo):
    global LAST_RESULT
    from concourse.bass_utils import run_bass_kernel_spmd

    if DTYPE not in _NC_CACHE:
        _NC_CACHE[DTYPE] = build_nc(dtype=DTYPE)
    nc = _NC_CACHE[DTYPE]

    in_maps = make_in_maps(query, key, value, Wq, bq, Wk, bk, Wv, bv, Wo, bo,
                           dtype=DTYPE)
    kwargs = {}
    if TRACE:
        kwargs = dict(trace=True, trace_cores=[0])
    res = run_bass_kernel_spmd(nc, in_maps, core_ids=list(range(N_CORES)),
                               **kwargs)
    LAST_RESULT = res

    out = np.zeros((B, S, D), np.float32)
    for core in range(N_CORES):
        b = core // GROUPS
        out[b] += res.results[core]["outT"].astype(np.float32).T
    return out
